# revision 6
# baseline (speedup 1.0000x reference)
"""Bass/Trainium2 kernel for nn_Attention_66297115181568 (sparse_attention).

Strategy: head-parallel across 8 NeuronCores; core h computes head h
end-to-end and its 64-row slice of the Wo projection. The host sums the
8 partial (512, 512) outputs (the tensor-parallel all-reduce) and adds bo.

v2 changes over the 76us baseline:
  1. rel-term matmuls run in fp8 DoubleRow perf mode (2 k-subtiles per
     pass, 2x moving-data ingest): moving is a [128, 2, 512] fp8 slice
     of the rel stream. PE rel ingest drops ~46.8us -> ~23.4us.
     DoubleRow is ISA-incompatible with PE column tiling (tile_position
     col != 0 fails walrus), so each (block, g-pair) stationary is a
     full 128-column [128, 2, 128] window that is zero outside its
     32-column block strip. The windows for the 4 blocks of a tile are
     stored OVERLAPPED in a [4*N] plane per (gp, t): block v's strip
     lives at X = v*160..v*160+32 so that window [v*128, v*128+128)
     contains exactly strip v (at column v*32+c) and zeros elsewhere.
     One memset + one strided tensor_mul per g writes all strips.
  2. q stays at natural scale in fp8 (3.6% quant err); the rel stream
     keeps its x64 host scale. The resulting x64 logit scale is folded
     into the k-RoPE constants (cosk/sink *= SCALE*64) and softmax runs
     exp(dots/64) via the activation scale - with NO max-subtraction
     (logits are O(1), exp is safe), removing 4 vector reduces.
  3. rel stream is 16 x 1MB pieces (one per 32-row i-block); each
     block's matmuls depend only on its own piece, so the PE tail after
     the last DMA byte is ~1.5us instead of a full 4MB chunk.
  4. Partial outputs written back in bf16 (halves write traffic); host
     accumulates in f32.
"""

import sys

sys.path.insert(0, "/opt/trn_rl_repo")

from contextlib import ExitStack

import numpy as np
import ml_dtypes

import concourse.bass as bass
import concourse.tile as tile
from concourse import mybir
from concourse.ap import AP

# problem dims (hardcoded per spec)
B, N, DIM, H, D = 1, 512, 512, 8, 64
INNER = H * D
N_CORES = 8
P = 128                 # SBUF partitions
NT = N // P             # 4 row tiles
KT = DIM // P           # 4 contraction tiles for projections
IB = 32                 # i-block rows (PE col-group granularity)
NB = N // IB            # 16 blocks
DG = 4                  # d's per d-group
NG = D // DG            # 16 d-groups
NGP = NG // 2           # 8 d-group pairs (DoubleRow)
SCALE = D ** -0.5
NEG_BIG = 1.0e36        # mask bias (scaled by RELSCALE must stay finite)
RELSCALE = 64.0         # host scales rel by this before fp8 cast

f32 = mybir.dt.float32
bf16 = mybir.dt.bfloat16
fp8 = mybir.dt.float8e4
AX = mybir.AxisListType
ALU = mybir.AluOpType
AF = mybir.ActivationFunctionType
PM = mybir.MatmulPerfMode


def legalize_multi_waits(nc):
    """This walrus build supports only one sync-wait per instruction; hoist
    extra waits onto same-engine NoOps placed immediately before."""
    nid = 0
    for fn in nc.m.functions:
        for bb in fn.blocks:
            new = []
            changed = False
            for inst in bb.instructions:
                si = inst.sync_info
                waits = si.on_wait if si is not None else []
                if len(waits) > 1:
                    for w in waits[:-1]:
                        nop = mybir.InstNoOp(name=f"I-waitfix-{nid}")
                        nid += 1
                        nop.engine = inst.engine
                        nop.sync_info = mybir.SyncInfo(on_wait=[w], on_update=[])
                        new.append(nop)
                    si.on_wait = [waits[-1]]
                    inst.sync_info = si
                    changed = True
                new.append(inst)
            if changed:
                bb.instructions = new


def build_nc(use_mask=True):
    nc = bass.Bass()

    # host pre-swizzles x^T and the weight slices into [partition, ...]
    # contiguous layouts so the input DMAs are clean 128-line transfers
    xt_ext = nc.declare_dram_parameter("xt", [P, KT * N], bf16, isOutput=False)
    wqks_ext = nc.declare_dram_parameter("wqks", [P, KT * 4 * D], bf16,
                                         isOutput=False)
    wv_ext = nc.declare_dram_parameter("wv", [P, KT * D], bf16, isOutput=False)
    # bias columns [D, 5] (q, k, qrot, krot, v) and the mask row
    biasc_ext = nc.declare_dram_parameter("biasc", [D, 5], f32, isOutput=False)
    maskrow_ext = nc.declare_dram_parameter("maskrow", [1, N], f32,
                                            isOutput=False)
    wo_ext = nc.declare_dram_parameter("wo", [D, DIM], bf16, isOutput=False)
    # cos/sin in transposed layout, [D, N] each: rows for q (plain) and for
    # k (scaled by D**-0.5 * RELSCALE), packed [D, 4, N]: cosq, cosk, sinq, sink
    cs_ext = nc.declare_dram_parameter("cs", [D, 4 * N], bf16, isOutput=False)
    tconst_ext = nc.declare_dram_parameter("tconst", [D, NG * P], bf16,
                                           isOutput=False)
    m512_ext = nc.declare_dram_parameter("m512", [P, N], bf16, isOutput=False)
    identf_ext = nc.declare_dram_parameter("identf", [P, P], f32, isOutput=False)
    # rel stream: [tile, p=(i_l*4+d_l), (block, g, j)] fp8; consumed as 1MB
    # pieces, one per (tile, block)
    rel_ext = nc.declare_dram_parameter("rel", [NT, P, NT * NG * N], fp8,
                                        isOutput=False)
    out_ext = nc.declare_dram_parameter("out", [N, DIM], bf16, isOutput=True)

    with tile.TileContext(nc) as tc, ExitStack() as ctx:
        dma = nc.sync      # HWDGE ring 1: inputs then the rel stream
        dma2 = nc.scalar   # HWDGE ring 2: output writebacks
        consts = ctx.enter_context(tc.tile_pool(name="consts", bufs=1))
        pro = ctx.enter_context(tc.tile_pool(name="pro", bufs=1))
        relp = ctx.enter_context(tc.tile_pool(name="relp", bufs=1))
        smp = ctx.enter_context(tc.tile_pool(name="smp", bufs=2))
        smallp = ctx.enter_context(tc.tile_pool(name="smallp", bufs=2))
        outp = ctx.enter_context(tc.tile_pool(name="outp", bufs=2))
        op = ctx.enter_context(tc.tile_pool(name="op", bufs=1))
        # PSUM: psA = proj (prologue) + dots (main); psB = rep (prologue);
        # psW = w/v transposes; psV = attnT accumulation; psO = out proj
        psA = ctx.enter_context(
            tc.tile_pool(name="psA", bufs=2, space=bass.MemorySpace.PSUM))
        psB = ctx.enter_context(
            tc.tile_pool(name="psB", bufs=2, space=bass.MemorySpace.PSUM))
        psW = ctx.enter_context(
            tc.tile_pool(name="psW", bufs=2, space=bass.MemorySpace.PSUM))
        psV = ctx.enter_context(
            tc.tile_pool(name="psV", bufs=1, space=bass.MemorySpace.PSUM))
        psO = ctx.enter_context(
            tc.tile_pool(name="psO", bufs=1, space=bass.MemorySpace.PSUM))

        # ---- small inputs FIRST on the sync ring (both HWDGE rings drain
        # as one FIFO stream: inputs must precede the rel pieces or the
        # prologue compute stalls behind the rel stream) ----
        xt_sb = pro.tile([P, KT, N], bf16)
        dma.dma_start(out=xt_sb[:], in_=xt_ext.rearrange("p (u n) -> p u n", u=KT))
        wqks_sb = pro.tile([P, KT, 4 * D], bf16)
        dma.dma_start(out=wqks_sb[:],
                      in_=wqks_ext.rearrange("p (u m) -> p u m", u=KT))
        wv_sb = pro.tile([P, KT, D], bf16)
        dma.dma_start(out=wv_sb[:], in_=wv_ext.rearrange("p (u m) -> p u m", u=KT))
        bias_cols = consts.tile([D, 5], f32)
        dma.dma_start(out=bias_cols[:], in_=biasc_ext[:])
        maskrow_sb = consts.tile([1, N], f32)
        if use_mask:
            dma.dma_start(out=maskrow_sb[:], in_=maskrow_ext[:])
        cs_sb = consts.tile([D, 4, N], bf16)
        dma.dma_start(out=cs_sb[:], in_=cs_ext.rearrange("d (c n) -> d c n", c=4))
        tconst_sb = consts.tile([D, NG, P], bf16)
        dma.dma_start(out=tconst_sb[:],
                      in_=tconst_ext.rearrange("d (g p) -> d g p", g=NG))
        m512_sb = consts.tile([P, N], bf16)
        dma.dma_start(out=m512_sb[:], in_=m512_ext[:])
        identf = consts.tile([P, P], f32)
        dma.dma_start(out=identf[:], in_=identf_ext[:])
        wo_sb = consts.tile([D, DIM], bf16)
        dma.dma_start(out=wo_sb[:], in_=wo_ext[:])
        ones_sb = consts.tile([1, N], f32)
        nc.vector.memset(ones_sb, 1.0)

        # ---- rel stream: 16 x 1MB pieces, serial on the sync ring, all
        # buffers resident. Piece (it, bl) covers i-rows
        # [it*128 + bl*32, +32) x all g x all j. ----
        rel_tiles = {}
        psz = NG * N           # bytes per partition per piece
        for it in range(NT):
            for bl in range(NT):
                rp = relp.tile([P, NG, N], fp8, name=f"rel{it}_{bl}")
                dma.dma_start(
                    out=rp.rearrange("p g j -> p (g j)")[:],
                    in_=rel_ext[it, :, bl * psz:(bl + 1) * psz])
                rel_tiles[(it, bl)] = rp

        # ---- projections (transposed): qT, kT, qrotT, krotT, vT ----
        qkT_sb = pro.tile([D, 4, N], bf16)
        for c in range(4):
            ps_c = psA.tile([P, N], f32, tag="big")
            for u in range(KT):
                nc.tensor.matmul(ps_c[0:D, :],
                                 wqks_sb[:, u, c * D:(c + 1) * D],
                                 xt_sb[:, u, :], start=(u == 0), stop=(u == KT - 1))
            nc.scalar.activation(qkT_sb[:, c, :], ps_c[0:D, :], AF.Identity,
                                 bias=bias_cols[:, c:c + 1])

        ps_v = psA.tile([P, N], f32, tag="big")
        for u in range(KT):
            nc.tensor.matmul(ps_v[0:D, :], wv_sb[:, u, :], xt_sb[:, u, :],
                             start=(u == 0), stop=(u == KT - 1))
        vT_sb = pro.tile([D, N], f32)
        nc.scalar.activation(vT_sb[:], ps_v[0:D, :], AF.Identity,
                             bias=bias_cols[:, 4:5])

        # ---- RoPE on DVE: q'T = cosq*qT + sinq*qrotT; k' likewise
        # (cosk/sink carry SCALE*RELSCALE) ----
        qkp_sb = pro.tile([D, 2, N], bf16)
        t1 = pro.tile([D, N], bf16, tag="ropet1")
        t2 = pro.tile([D, N], bf16, tag="ropet2")
        for c in range(2):  # 0: q, 1: k
            nc.vector.tensor_mul(t1[:], qkT_sb[:, c, :], cs_sb[:, c, :])
            nc.vector.tensor_mul(t2[:], qkT_sb[:, 2 + c, :], cs_sb[:, 2 + c, :])
            nc.vector.tensor_add(qkp_sb[:, c, :], t1[:], t2[:])
        qpT = qkp_sb[:, 0, :]
        kpT = qkp_sb[:, 1, :]

        # ---- v -> [j, d] layout via PE transposes ----
        v_sb = pro.tile([P, NT, D], bf16)
        for jt in range(NT):
            pv = psW.tile([P, P], f32, tag="tp")
            nc.tensor.transpose(pv[:, 0:D], vT_sb[:, jt * P:(jt + 1) * P],
                                identf[0:D, 0:D])
            nc.scalar.copy(v_sb[:, jt, :], pv[:, 0:D])

        # ---- Qpad stationaries (fp8, natural q scale): for g = 2gp + t,
        # Rep_g[p, n] = q'T[g*4 + p%4, n]; the block-diag-masked values are
        # scattered into overlapped zero-padded planes: plane (gp, t) is
        # [NT, N] where block v of tile u lives at [u, v*160 + c] (c = p//4
        # column within the strip). The DoubleRow stationary for (it, bl,
        # gp) is then the plain window [:, gp, :, it, bl*128 : bl*128+128].
        qpad = consts.tile([P, NGP, 2, NT, N], fp8, name="qpad")
        nc.vector.memset(qpad, 0.0)
        qpad_full = qpad[:]
        ppair = list(qpad_full.ap[0])
        for g in range(NG):
            ps_rep = psB.tile([P, N], f32, tag="rep")
            nc.tensor.matmul(ps_rep[:], tconst_sb[:, g, :], qpT,
                             start=True, stop=True)
            # strided strip write: out[p, u, v, c] -> qpad plane offset
            # u*N + v*160 + c; in = (ps_rep * m512)[p, u*128 + v*32 + c]
            strip_out = AP(qpad_full.tensor,
                           qpad_full.offset + g * (NT * N),
                           [ppair, [N, NT], [160, NT], [1, IB]])
            nc.vector.tensor_mul(
                strip_out,
                ps_rep.rearrange("p (u v c) -> p u v c", u=NT, v=NT)[:],
                m512_sb.rearrange("p (u v c) -> p u v c", u=NT, v=NT)[:])

        # ---- main loop over row tiles ----
        o_tiles = [op.tile([P, DIM], bf16, name=f"o{it}") for it in range(NT)]
        for it in range(NT):
            dots_ps = psA.tile([P, N], f32, tag="big")
            # QK^T logits first (start=True resets the full bank)
            nc.tensor.matmul(dots_ps[:], qpT[:, it * P:(it + 1) * P], kpT,
                             start=True, stop=False, skip_group_check=True)
            if use_mask:
                # mask bias row (additive, 0 kept / -BIG*RELSCALE masked)
                nc.tensor.matmul(dots_ps[:], ones_sb[:, 0:P], maskrow_sb[:],
                                 start=False, stop=False, skip_group_check=True)
            # rel term: per 32-row block, 8 DoubleRow fp8 matmuls (one per
            # g-pair); each block depends only on its own 1MB rel piece.
            # Full 128-col stationary window (zero outside the block strip)
            # accumulating into the full bank - DoubleRow forbids PE column
            # tiling, and adding zeros elsewhere is a no-op.
            for bl in range(NT):
                rp = rel_tiles[(it, bl)]
                for gp in range(NGP):
                    nc.tensor.matmul(
                        dots_ps[:],
                        qpad[:, gp, :, it, bl * P:(bl + 1) * P],
                        rp[:, 2 * gp:2 * gp + 2, :],
                        start=False, stop=(bl == NT - 1 and gp == NGP - 1),
                        perf_mode=PM.DoubleRow,
                        skip_group_check=True)

            # softmax: unnormalized exp(dots/RELSCALE), no max-subtraction
            # (logits are O(1)); 1/rowsum folded into the output copy
            w_sm = smp.tile([P, N], f32, tag="w_sm")
            rowsum = smallp.tile([P, 1], f32, tag="rowsum")
            nc.scalar.activation(w_sm[:], dots_ps[:], AF.Exp,
                                 scale=1.0 / RELSCALE, accum_out=rowsum[:])
            rcp = smallp.tile([P, 1], f32, tag="rcp")
            nc.vector.reciprocal(rcp[:], rowsum[:])

            wT_sb = outp.tile([P, NT, P], bf16, tag="wT_sb")
            for jt in range(NT):
                wp = psW.tile([P, P], f32, tag="tp")
                nc.tensor.transpose(wp[:], w_sm[:, jt * P:(jt + 1) * P], identf[:])
                nc.scalar.copy(wT_sb[:, jt, :], wp[:])

            attn_ps = psV.tile([D, P], f32, tag="attn")
            for jt in range(NT):
                nc.tensor.matmul(attn_ps[:], v_sb[:, jt, :], wT_sb[:, jt, :],
                                 start=(jt == 0), stop=(jt == NT - 1))
            attn_sb = outp.tile([D, P], bf16, tag="attn_sb")
            nc.scalar.copy(attn_sb[:], attn_ps[:])

            out_ps = psO.tile([P, DIM], f32, tag="out")
            nc.tensor.matmul(out_ps[:], attn_sb[:], wo_sb[:], start=True, stop=True)
            nc.scalar.activation(o_tiles[it][:], out_ps[:], AF.Copy, scale=rcp[:])
            dma2.dma_start(out=out_ext[it * P:(it + 1) * P, :], in_=o_tiles[it][:])

    legalize_multi_waits(nc)
    return nc


_NC_CACHE = None
TRACE = False        # set by test harness to capture an NTFF profile
LAST_RESULT = None   # BassKernelResults of the most recent kernel() call


def _get_nc(use_mask):
    global _NC_CACHE
    if _NC_CACHE is None or _NC_CACHE[1] != use_mask:
        _NC_CACHE = (build_nc(use_mask), use_mask)
    return _NC_CACHE[0]


def _rot_mat():
    """rotate_half as a right-multiply matrix: rot(q) = q @ Rm."""
    Rm = np.zeros((D, D), np.float32)
    for i in range(D // 2):
        Rm[2 * i + 1, 2 * i] = -1.0
        Rm[2 * i, 2 * i + 1] = 1.0
    return Rm


def kernel(**inputs):
    x = np.asarray(inputs["x"], dtype=np.float32)
    mask = np.asarray(inputs["mask"])
    rope = np.asarray(inputs["rope"], dtype=np.float32)
    rel_pos = np.asarray(inputs["rel_pos"], dtype=np.float32)
    Wq = np.asarray(inputs["Wq"], dtype=np.float32)
    bq = np.asarray(inputs["bq"], dtype=np.float32)
    Wk = np.asarray(inputs["Wk"], dtype=np.float32)
    bk = np.asarray(inputs["bk"], dtype=np.float32)
    Wv = np.asarray(inputs["Wv"], dtype=np.float32)
    bv = np.asarray(inputs["bv"], dtype=np.float32)
    Wo = np.asarray(inputs["Wo"], dtype=np.float32)
    bo = np.asarray(inputs["bo"], dtype=np.float32)

    use_mask = not bool(np.asarray(mask).all())
    nc = _get_nc(use_mask)
    Rm = _rot_mat()

    def swz(a):  # [K, M] -> [p, (u, M)] with K = (u, p)
        k, m = a.shape
        return np.ascontiguousarray(
            a.reshape(KT, P, m).transpose(1, 0, 2).reshape(P, KT * m))

    xT = swz(x.reshape(N, DIM).T.astype(np.float32)).astype(ml_dtypes.bfloat16)
    maskrow = ((mask.reshape(1, N).astype(np.float32)) - 1.0) * NEG_BIG

    # cos/sin in transposed layout, packed [D, 4*N]: cosq, cosk, sinq, sink
    # (k columns carry the QK scale and the RELSCALE logit scale)
    cosT = np.cos(rope).T.astype(np.float32)      # [D, N]
    sinT = np.sin(rope).T.astype(np.float32)
    kf = SCALE * RELSCALE
    cs = np.concatenate([cosT, cosT * kf, sinT, sinT * kf],
                        axis=1).astype(ml_dtypes.bfloat16)

    # T[d, g, p] = (d == g*4 + p%4); m512[p, n] = (n%32 == p//4)
    d_i = np.arange(D)[:, None, None]
    g_i = np.arange(NG)[None, :, None]
    p_i = np.arange(P)[None, None, :]
    tconst = (d_i == g_i * DG + p_i % DG).astype(np.float32)
    tconst = tconst.reshape(D, NG * P).astype(ml_dtypes.bfloat16)
    p_2 = np.arange(P)[:, None]
    n_2 = np.arange(N)[None, :]
    m512 = ((n_2 % IB) == (p_2 // DG)).astype(np.float32)
    m512 = m512.astype(ml_dtypes.bfloat16)

    identf = np.eye(P, dtype=np.float32)

    # rel chunks: [h, c, p=(i_l*4+d_l), (b4, g, j)] fp8, scaled by RELSCALE
    rel8 = (rel_pos[0] * RELSCALE).astype(ml_dtypes.float8_e4m3)
    # [h, (c, b4, i_l), j, (g, d_l)] -> [h, c, i_l, d_l, b4, g, j]
    rel8 = rel8.reshape(H, NT, NT, IB, N, NG, DG)
    rel8 = np.ascontiguousarray(rel8.transpose(0, 1, 3, 6, 2, 5, 4))
    rel8 = rel8.reshape(H, NT, P, NT * NG * N)

    in_maps = []
    for h in range(N_CORES):
        sl = slice(h * D, (h + 1) * D)
        wq, wk = Wq[:, sl], Wk[:, sl]
        wqks = np.concatenate([wq, wk, wq @ Rm, wk @ Rm], axis=1)
        biasc = np.stack([bq[sl], bk[sl], bq[sl] @ Rm, bk[sl] @ Rm,
                          bv[sl]], axis=1).astype(np.float32)
        in_maps.append({
            "xt": xT,
            "wqks": swz(wqks).astype(ml_dtypes.bfloat16),
            "wv": swz(np.ascontiguousarray(Wv[:, sl])).astype(ml_dtypes.bfloat16),
            "biasc": np.ascontiguousarray(biasc),
            "maskrow": np.ascontiguousarray(maskrow),
            "wo": np.ascontiguousarray(Wo[sl, :]).astype(ml_dtypes.bfloat16),
            "cs": cs,
            "tconst": tconst,
            "m512": m512,
            "identf": identf,
            "rel": rel8[h],
        })

    from concourse.bass_utils import run_bass_kernel_spmd
    res = run_bass_kernel_spmd(nc, in_maps, list(range(N_CORES)), trace=TRACE)
    globals()["LAST_RESULT"] = res
    out = np.zeros((N, DIM), dtype=np.float32)
    for h in range(N_CORES):
        out += np.asarray(res.results[h]["out"], dtype=np.float32)
    out += bo[None, :]
    return out.reshape(B, N, DIM)


# revision 7
# speedup vs baseline: 1.1426x; 1.1426x over previous
"""Bass/Trainium2 kernel for nn_Attention_66297115181568 (sparse_attention).

Strategy: head-parallel across 8 NeuronCores; core h computes head h
end-to-end and its 64-row slice of the Wo projection. The host sums the
8 partial (512, 512) outputs (the tensor-parallel all-reduce) and adds bo.

v3 (from the 76us v1 baseline and the 102us v2 experiment):
  1. rel-term matmuls in fp8 DoubleRow perf mode: moving [128, 2, 512]
     fp8 rel slices stream 2 fp8/partition/cycle (measured 216ns per
     matmul vs 427ns bf16-equivalent). DoubleRow is ISA-incompatible
     with PE column tiling, so each (block, g-pair) stationary is a
     full 128-column [128, 2, 128] window, zero outside its 32-column
     block strip. Windows for the 4 blocks of a tile are OVERLAPPED in
     a [4*N] plane per (gp, t): block v's strip lives at X=v*160..+32,
     so window [v*128, v*128+128) holds exactly strip v, zeros
     elsewhere. One strided tensor_mul per g writes all 16 strips.
  2. The qpad zero-fill (32KB/partition) is split between GPSIMD and
     DVE so it is off the DVE critical path (v2 lesson: a single 27us
     DVE memset serialized RoPE/strip-muls behind it -> rel matmuls
     started at 62us).
  3. DMA: inputs + even rel pieces on the sync HWDGE queue; odd pieces
     + outputs on the scalar queue (probes 2-queue aggregate BW; the
     single queue measured 423 GB/s). rel is 16 x 1MB pieces, one per
     32-row i-block; the final piece is split in two 512KB halves so
     the post-DMA PE tail is ~1us.
  4. Softmax: exp((qk*64 + rel*64)/64) with NO max-subtraction (logits
     are O(1)); the x64 logit scale is folded into the k-projection
     activation scale (cs carries only plain cos/sin rows). Exp runs
     in 4 column chunks so w-transposes/attnV pipeline behind it;
     rowsums accumulate per-chunk then reduce.
  5. Partial outputs written back bf16; host accumulates in f32.
"""

import sys

sys.path.insert(0, "/opt/trn_rl_repo")

from contextlib import ExitStack

import numpy as np
import ml_dtypes

import concourse.bass as bass
import concourse.tile as tile
from concourse import mybir
from concourse.ap import AP

# problem dims (hardcoded per spec)
B, N, DIM, H, D = 1, 512, 512, 8, 64
INNER = H * D
N_CORES = 8
P = 128                 # SBUF partitions
NT = N // P             # 4 row tiles
KT = DIM // P           # 4 contraction tiles for projections
IB = 32                 # i-block rows
NB = N // IB            # 16 blocks
DG = 4                  # d's per d-group
NG = D // DG            # 16 d-groups
NGP = NG // 2           # 8 d-group pairs (DoubleRow)
SCALE = D ** -0.5
NEG_BIG = 1.0e36
RELSCALE = 64.0         # host scales rel by this before fp8 cast
KF = SCALE * RELSCALE   # folded into the k/krot projection activation

f32 = mybir.dt.float32
bf16 = mybir.dt.bfloat16
fp8 = mybir.dt.float8e4
AX = mybir.AxisListType
ALU = mybir.AluOpType
AF = mybir.ActivationFunctionType
PM = mybir.MatmulPerfMode


def legalize_multi_waits(nc):
    """This walrus build supports only one sync-wait per instruction; hoist
    extra waits onto same-engine NoOps placed immediately before."""
    nid = 0
    for fn in nc.m.functions:
        for bb in fn.blocks:
            new = []
            changed = False
            for inst in bb.instructions:
                si = inst.sync_info
                waits = si.on_wait if si is not None else []
                if len(waits) > 1:
                    for w in waits[:-1]:
                        nop = mybir.InstNoOp(name=f"I-waitfix-{nid}")
                        nid += 1
                        nop.engine = inst.engine
                        nop.sync_info = mybir.SyncInfo(on_wait=[w], on_update=[])
                        new.append(nop)
                    si.on_wait = [waits[-1]]
                    inst.sync_info = si
                    changed = True
                new.append(inst)
            if changed:
                bb.instructions = new


def build_nc(use_mask=True):
    nc = bass.Bass()

    xt_ext = nc.declare_dram_parameter("xt", [P, KT * N], bf16, isOutput=False)
    wqks_ext = nc.declare_dram_parameter("wqks", [P, KT * 4 * D], bf16,
                                         isOutput=False)
    wv_ext = nc.declare_dram_parameter("wv", [P, KT * D], bf16, isOutput=False)
    biasc_ext = nc.declare_dram_parameter("biasc", [D, 5], f32, isOutput=False)
    maskrow_ext = nc.declare_dram_parameter("maskrow", [1, N], f32,
                                            isOutput=False)
    wo_ext = nc.declare_dram_parameter("wo", [D, DIM], bf16, isOutput=False)
    # plain cos/sin in transposed layout [D, 2, N] (no k scaling here)
    cs_ext = nc.declare_dram_parameter("cs", [D, 2 * N], bf16, isOutput=False)
    tconst_ext = nc.declare_dram_parameter("tconst", [D, NG * P], fp8,
                                           isOutput=False)
    m512_ext = nc.declare_dram_parameter("m512", [P, N], bf16, isOutput=False)
    identf_ext = nc.declare_dram_parameter("identf", [P, P], f32, isOutput=False)
    # rel stream: [tile, p=(i_l*4+d_l), (block, g, j)] fp8; 1MB pieces
    rel_ext = nc.declare_dram_parameter("rel", [NT, P, NT * NG * N], fp8,
                                        isOutput=False)
    out_ext = nc.declare_dram_parameter("out", [N, DIM], bf16, isOutput=True)

    with tile.TileContext(nc) as tc, ExitStack() as ctx:
        dma = nc.sync      # HWDGE queue 1: inputs + even rel pieces
        dma2 = nc.scalar   # HWDGE queue 2: odd rel pieces + outputs
        consts = ctx.enter_context(tc.tile_pool(name="consts", bufs=1))
        pro = ctx.enter_context(tc.tile_pool(name="pro", bufs=1))
        relp = ctx.enter_context(tc.tile_pool(name="relp", bufs=1))
        smp = ctx.enter_context(tc.tile_pool(name="smp", bufs=2))
        smallp = ctx.enter_context(tc.tile_pool(name="smallp", bufs=2))
        outp = ctx.enter_context(tc.tile_pool(name="outp", bufs=2))
        op = ctx.enter_context(tc.tile_pool(name="op", bufs=1))
        psA = ctx.enter_context(
            tc.tile_pool(name="psA", bufs=2, space=bass.MemorySpace.PSUM))
        psB = ctx.enter_context(
            tc.tile_pool(name="psB", bufs=2, space=bass.MemorySpace.PSUM))
        psW = ctx.enter_context(
            tc.tile_pool(name="psW", bufs=2, space=bass.MemorySpace.PSUM))
        psV = ctx.enter_context(
            tc.tile_pool(name="psV", bufs=1, space=bass.MemorySpace.PSUM))
        psO = ctx.enter_context(
            tc.tile_pool(name="psO", bufs=1, space=bass.MemorySpace.PSUM))

        # ---- qpad zero-fill FIRST, split gpsimd/DVE (both idle at t0) ----
        qpad = consts.tile([P, NGP, 2, NT, N], fp8, name="qpad")
        nc.gpsimd.memset(qpad[:, 0:NGP // 2], 0.0)
        nc.vector.memset(qpad[:, NGP // 2:NGP], 0.0)

        # ---- inputs on queue 1 (ahead of its rel pieces) ----
        xt_sb = pro.tile([P, KT, N], bf16)
        dma.dma_start(out=xt_sb[:], in_=xt_ext.rearrange("p (u n) -> p u n", u=KT))
        wqks_sb = pro.tile([P, KT, 4 * D], bf16)
        dma.dma_start(out=wqks_sb[:],
                      in_=wqks_ext.rearrange("p (u m) -> p u m", u=KT))
        wv_sb = pro.tile([P, KT, D], bf16)
        dma.dma_start(out=wv_sb[:], in_=wv_ext.rearrange("p (u m) -> p u m", u=KT))
        bias_cols = consts.tile([D, 5], f32)
        dma.dma_start(out=bias_cols[:], in_=biasc_ext[:])
        maskrow_sb = consts.tile([1, N], f32)
        if use_mask:
            dma.dma_start(out=maskrow_sb[:], in_=maskrow_ext[:])
        cs_sb = consts.tile([D, 2, N], bf16)
        dma.dma_start(out=cs_sb[:], in_=cs_ext.rearrange("d (c n) -> d c n", c=2))
        tconst_sb = consts.tile([D, NG, P], fp8)
        dma.dma_start(out=tconst_sb[:],
                      in_=tconst_ext.rearrange("d (g p) -> d g p", g=NG))
        m512_sb = consts.tile([P, N], bf16)
        dma.dma_start(out=m512_sb[:], in_=m512_ext[:])
        identf = consts.tile([P, P], f32)
        dma.dma_start(out=identf[:], in_=identf_ext[:])
        wo_sb = consts.tile([D, DIM], bf16)
        dma.dma_start(out=wo_sb[:], in_=wo_ext[:])
        ones_sb = consts.tile([1, N], f32)
        nc.vector.memset(ones_sb, 1.0)

        # ---- rel stream: 16 x 1MB pieces, alternating queues; last piece
        # split into two 512KB halves so the PE tail after the final DMA
        # byte is short. ----
        rel_tiles = {}
        psz = NG * N
        hsz = (NG // 2) * N
        for it in range(NT):
            for bl in range(NT):
                k = it * NT + bl
                q = dma if k % 2 == 0 else dma2
                if (it, bl) == (NT - 1, NT - 1):
                    rpa = relp.tile([P, NG // 2, N], fp8, name="rel15a")
                    dma.dma_start(out=rpa.rearrange("p g j -> p (g j)")[:],
                                  in_=rel_ext[it, :, bl * psz:bl * psz + hsz])
                    rpb = relp.tile([P, NG // 2, N], fp8, name="rel15b")
                    dma2.dma_start(out=rpb.rearrange("p g j -> p (g j)")[:],
                                   in_=rel_ext[it, :, bl * psz + hsz:(bl + 1) * psz])
                    rel_tiles[(it, bl)] = (rpa, rpb)
                else:
                    rp = relp.tile([P, NG, N], fp8, name=f"rel{it}_{bl}")
                    q.dma_start(out=rp.rearrange("p g j -> p (g j)")[:],
                                in_=rel_ext[it, :, bl * psz:(bl + 1) * psz])
                    rel_tiles[(it, bl)] = rp

        # ---- projections (transposed): qT, kT, qrotT, krotT, vT ----
        # k/krot activations carry the KF = SCALE*RELSCALE logit scale
        qkT_sb = pro.tile([D, 4, N], bf16)
        for c in range(4):
            ps_c = psA.tile([P, N], f32, tag="big")
            for u in range(KT):
                nc.tensor.matmul(ps_c[0:D, :],
                                 wqks_sb[:, u, c * D:(c + 1) * D],
                                 xt_sb[:, u, :], start=(u == 0), stop=(u == KT - 1))
            nc.scalar.activation(qkT_sb[:, c, :], ps_c[0:D, :], AF.Identity,
                                 bias=bias_cols[:, c:c + 1],
                                 scale=(KF if c in (1, 3) else 1.0))

        ps_v = psA.tile([P, N], f32, tag="big")
        for u in range(KT):
            nc.tensor.matmul(ps_v[0:D, :], wv_sb[:, u, :], xt_sb[:, u, :],
                             start=(u == 0), stop=(u == KT - 1))
        vT_sb = pro.tile([D, N], f32)
        nc.scalar.activation(vT_sb[:], ps_v[0:D, :], AF.Identity,
                             bias=bias_cols[:, 4:5])

        # ---- RoPE on DVE: q'T = cos*qT + sin*qrotT; k' likewise ----
        qkp_sb = pro.tile([D, 2, N], bf16)
        t1 = pro.tile([D, N], bf16, tag="ropet1")
        t2 = pro.tile([D, N], bf16, tag="ropet2")
        for c in range(2):  # 0: q, 1: k
            nc.vector.tensor_mul(t1[:], qkT_sb[:, c, :], cs_sb[:, 0, :])
            nc.vector.tensor_mul(t2[:], qkT_sb[:, 2 + c, :], cs_sb[:, 1, :])
            nc.vector.tensor_add(qkp_sb[:, c, :], t1[:], t2[:])
        qpT = qkp_sb[:, 0, :]
        kpT = qkp_sb[:, 1, :]

        # ---- v -> [j, d] layout via PE transposes ----
        v_sb = pro.tile([P, NT, D], bf16)
        for jt in range(NT):
            pv = psW.tile([P, P], f32, tag="tp")
            nc.tensor.transpose(pv[:, 0:D], vT_sb[:, jt * P:(jt + 1) * P],
                                identf[0:D, 0:D])
            nc.scalar.copy(v_sb[:, jt, :], pv[:, 0:D])

        # ---- Qpad stationaries: Rep_g[p, n] = q'T[g*4 + p%4, n]; the
        # block-diag-masked values go into overlapped zero-padded planes:
        # plane (gp, t) is [NT, N]; block v of tile u lives at
        # [u, v*160 + c] (c = p//4). One strided tensor_mul per g. ----
        qpad_full = qpad[:]
        ppair = list(qpad_full.ap[0])
        for g in range(NG):
            ps_rep = psB.tile([P, N], f32, tag="rep")
            nc.tensor.matmul(ps_rep[:], tconst_sb[:, g, :], qpT,
                             start=True, stop=True)
            strip_out = AP(qpad_full.tensor,
                           qpad_full.offset + g * (NT * N),
                           [ppair, [N, NT], [IB * 5, NT], [1, IB]])
            nc.vector.tensor_mul(
                strip_out,
                ps_rep.rearrange("p (u v c) -> p u v c", u=NT, v=NT)[:],
                m512_sb.rearrange("p (u v c) -> p u v c", u=NT, v=NT)[:])

        # ---- main loop over row tiles ----
        o_tiles = [op.tile([P, DIM], bf16, name=f"o{it}") for it in range(NT)]
        for it in range(NT):
            dots_ps = psA.tile([P, N], f32, tag="big")
            nc.tensor.matmul(dots_ps[:], qpT[:, it * P:(it + 1) * P], kpT,
                             start=True, stop=False, skip_group_check=True)
            if use_mask:
                nc.tensor.matmul(dots_ps[:], ones_sb[:, 0:P], maskrow_sb[:],
                                 start=False, stop=False, skip_group_check=True)
            for bl in range(NT):
                rp = rel_tiles[(it, bl)]
                last = (bl == NT - 1)
                for gp in range(NGP):
                    if isinstance(rp, tuple):
                        rph = rp[0] if gp < NGP // 2 else rp[1]
                        mv = rph[:, 2 * (gp % (NGP // 2)):2 * (gp % (NGP // 2)) + 2, :]
                    else:
                        mv = rp[:, 2 * gp:2 * gp + 2, :]
                    nc.tensor.matmul(
                        dots_ps[:],
                        qpad[:, gp, :, it, bl * P:(bl + 1) * P],
                        mv,
                        start=False, stop=(last and gp == NGP - 1),
                        perf_mode=PM.DoubleRow,
                        skip_group_check=True)

            # softmax: unnormalized exp(dots/RELSCALE), no max-subtraction;
            # 4 column chunks so transposes/attnV pipeline behind exp
            w_sm = smp.tile([P, N], f32, tag="w_sm")
            rowsum4 = smallp.tile([P, NT], f32, tag="rowsum4")
            wT_sb = outp.tile([P, NT, P], bf16, tag="wT_sb")
            attn_ps = psV.tile([D, P], f32, tag="attn")
            for jt in range(NT):
                nc.scalar.activation(w_sm[:, jt * P:(jt + 1) * P],
                                     dots_ps[:, jt * P:(jt + 1) * P], AF.Exp,
                                     scale=1.0 / RELSCALE,
                                     accum_out=rowsum4[:, jt:jt + 1])
                wp = psW.tile([P, P], f32, tag="tp")
                nc.tensor.transpose(wp[:], w_sm[:, jt * P:(jt + 1) * P], identf[:])
                nc.vector.tensor_copy(wT_sb[:, jt, :], wp[:])
                nc.tensor.matmul(attn_ps[:], v_sb[:, jt, :], wT_sb[:, jt, :],
                                 start=(jt == 0), stop=(jt == NT - 1))
            rowsum = smallp.tile([P, 1], f32, tag="rowsum")
            nc.vector.tensor_reduce(rowsum[:], rowsum4[:], AX.X, ALU.add)
            rcp = smallp.tile([P, 1], f32, tag="rcp")
            nc.vector.reciprocal(rcp[:], rowsum[:])

            attn_sb = outp.tile([D, P], bf16, tag="attn_sb")
            nc.scalar.copy(attn_sb[:], attn_ps[:])
            out_ps = psO.tile([P, DIM], f32, tag="out")
            nc.tensor.matmul(out_ps[:], attn_sb[:], wo_sb[:], start=True, stop=True)
            nc.scalar.activation(o_tiles[it][:], out_ps[:], AF.Copy, scale=rcp[:])
            dma2.dma_start(out=out_ext[it * P:(it + 1) * P, :], in_=o_tiles[it][:])

    legalize_multi_waits(nc)
    return nc


_NC_CACHE = None
TRACE = False
LAST_RESULT = None


def _get_nc(use_mask):
    global _NC_CACHE
    if _NC_CACHE is None or _NC_CACHE[1] != use_mask:
        _NC_CACHE = (build_nc(use_mask), use_mask)
    return _NC_CACHE[0]


def _rot_mat():
    """rotate_half as a right-multiply matrix: rot(q) = q @ Rm."""
    Rm = np.zeros((D, D), np.float32)
    for i in range(D // 2):
        Rm[2 * i + 1, 2 * i] = -1.0
        Rm[2 * i, 2 * i + 1] = 1.0
    return Rm


def kernel(**inputs):
    x = np.asarray(inputs["x"], dtype=np.float32)
    mask = np.asarray(inputs["mask"])
    rope = np.asarray(inputs["rope"], dtype=np.float32)
    rel_pos = np.asarray(inputs["rel_pos"], dtype=np.float32)
    Wq = np.asarray(inputs["Wq"], dtype=np.float32)
    bq = np.asarray(inputs["bq"], dtype=np.float32)
    Wk = np.asarray(inputs["Wk"], dtype=np.float32)
    bk = np.asarray(inputs["bk"], dtype=np.float32)
    Wv = np.asarray(inputs["Wv"], dtype=np.float32)
    bv = np.asarray(inputs["bv"], dtype=np.float32)
    Wo = np.asarray(inputs["Wo"], dtype=np.float32)
    bo = np.asarray(inputs["bo"], dtype=np.float32)

    use_mask = not bool(np.asarray(mask).all())
    nc = _get_nc(use_mask)
    Rm = _rot_mat()

    def swz(a):  # [K, M] -> [p, (u, M)] with K = (u, p)
        k, m = a.shape
        return np.ascontiguousarray(
            a.reshape(KT, P, m).transpose(1, 0, 2).reshape(P, KT * m))

    xT = swz(x.reshape(N, DIM).T.astype(np.float32)).astype(ml_dtypes.bfloat16)
    maskrow = ((mask.reshape(1, N).astype(np.float32)) - 1.0) * NEG_BIG

    # plain cos/sin, packed [D, 2*N]
    cosT = np.cos(rope).T.astype(np.float32)
    sinT = np.sin(rope).T.astype(np.float32)
    cs = np.concatenate([cosT, sinT], axis=1).astype(ml_dtypes.bfloat16)

    # T[d, g, p] = (d == g*4 + p%4); m512[p, n] = (n%32 == p//4)
    d_i = np.arange(D)[:, None, None]
    g_i = np.arange(NG)[None, :, None]
    p_i = np.arange(P)[None, None, :]
    tconst = (d_i == g_i * DG + p_i % DG).astype(np.float32)
    tconst = tconst.reshape(D, NG * P).astype(ml_dtypes.float8_e4m3)
    p_2 = np.arange(P)[:, None]
    n_2 = np.arange(N)[None, :]
    m512 = ((n_2 % IB) == (p_2 // DG)).astype(np.float32)
    m512 = m512.astype(ml_dtypes.bfloat16)

    identf = np.eye(P, dtype=np.float32)

    # rel chunks: [h, c, p=(i_l*4+d_l), (b4, g, j)] fp8, scaled by RELSCALE
    rel8 = (rel_pos[0] * RELSCALE).astype(ml_dtypes.float8_e4m3)
    rel8 = rel8.reshape(H, NT, NT, IB, N, NG, DG)
    rel8 = np.ascontiguousarray(rel8.transpose(0, 1, 3, 6, 2, 5, 4))
    rel8 = rel8.reshape(H, NT, P, NT * NG * N)

    in_maps = []
    for h in range(N_CORES):
        sl = slice(h * D, (h + 1) * D)
        wq, wk = Wq[:, sl], Wk[:, sl]
        wqks = np.concatenate([wq, wk, wq @ Rm, wk @ Rm], axis=1)
        biasc = np.stack([bq[sl], bk[sl] * KF, bq[sl] @ Rm,
                          (bk[sl] @ Rm) * KF, bv[sl]], axis=1).astype(np.float32)
        in_maps.append({
            "xt": xT,
            "wqks": swz(wqks).astype(ml_dtypes.bfloat16),
            "wv": swz(np.ascontiguousarray(Wv[:, sl])).astype(ml_dtypes.bfloat16),
            "biasc": np.ascontiguousarray(biasc),
            "maskrow": np.ascontiguousarray(maskrow),
            "wo": np.ascontiguousarray(Wo[sl, :]).astype(ml_dtypes.bfloat16),
            "cs": cs,
            "tconst": tconst,
            "m512": m512,
            "identf": identf,
            "rel": rel8[h],
        })

    from concourse.bass_utils import run_bass_kernel_spmd
    res = run_bass_kernel_spmd(nc, in_maps, list(range(N_CORES)), trace=TRACE)
    globals()["LAST_RESULT"] = res
    out = np.zeros((N, DIM), dtype=np.float32)
    for h in range(N_CORES):
        out += np.asarray(res.results[h]["out"], dtype=np.float32)
    out += bo[None, :]
    return out.reshape(B, N, DIM)


# revision 8
# speedup vs baseline: 1.2397x; 1.0850x over previous
"""Bass/Trainium2 kernel for nn_Attention_66297115181568 (sparse_attention).

Strategy: head-parallel across 8 NeuronCores; core h computes head h
end-to-end and its 64-row slice of the Wo projection. The host sums the
8 partial (512, 512) outputs (the tensor-parallel all-reduce) and adds bo.

v4 (lessons from v1=76us, v2=102us, v3=90us):
  1. rel-term matmuls in fp8 DoubleRow perf mode (measured 216ns per
     [128, 2, 512] matmul at cruise = 2 fp8/partition/cycle, 2x the
     bf16 col-tiled scheme). DoubleRow is ISA-incompatible with PE
     column tiling, so each (block, g-pair) stationary is a full
     128-column [128, 2, 128] window, zero outside its block strip.
  2. IB=8 block geometry (partitions = 8 rows x 16 d): only NG=4
     d-groups -> 4 repl matmuls + 4 strip-muls (v3 had 16+16, a ~13us
     serial prologue chain). Windows for the 16 blocks of a tile are
     overlapped in a [1928]-elem plane per (gp, t, u): window bl starts
     at 120*bl, strip bl at 128*bl, so each window holds exactly its
     own strip (at column 8*bl+c) and shared zeros elsewhere.
  3. qpad zero-fill as f32-bitcast memsets (4B/cycle instead of 1),
     split gpsimd/DVE, emitted first.
  4. DMA: ALL 16 1MB rel pieces on the sync queue (423 GB/s measured;
     v3's split put pieces behind main-loop ACT doorbells -> 87us
     stragglers). Inputs on the scalar queue, emitted first, so the rel
     stream starts at t~9us (queue-up latency) with no input prefix.
     Outputs (bf16) also on the scalar queue.
  5. tile 0 consumes g-pairs as they are built (repl/strip/matmul
     interleave) instead of waiting for all strips.
  6. Softmax: exp(dots/64) with no max-subtraction; the x64 logit scale
     rides in the k-projection activation (KF); exp in 4 column chunks
     so w-transposes/attnV pipeline behind it.
"""

import sys

sys.path.insert(0, "/opt/trn_rl_repo")

from contextlib import ExitStack

import numpy as np
import ml_dtypes

import concourse.bass as bass
import concourse.tile as tile
from concourse import mybir
from concourse.ap import AP

# problem dims (hardcoded per spec)
B, N, DIM, H, D = 1, 512, 512, 8, 64
INNER = H * D
N_CORES = 8
P = 128                 # SBUF partitions
NT = N // P             # 4 row tiles
KT = DIM // P           # 4 contraction tiles for projections
IB = 8                  # i-block rows per partition-group
NBT = P // IB           # 16 blocks per row tile
DG = P // IB            # 16 d's per d-group
NG = D // DG            # 4 d-groups
NGP = NG // 2           # 2 d-group pairs (DoubleRow)
WS = P - IB             # 120: window stride in the packed plane
PW = WS * (NBT - 1) + P  # 1928: packed plane width
SCALE = D ** -0.5
NEG_BIG = 1.0e36
RELSCALE = 64.0         # host scales rel by this before fp8 cast
KF = SCALE * RELSCALE   # folded into the k/krot projection activation

f32 = mybir.dt.float32
bf16 = mybir.dt.bfloat16
fp8 = mybir.dt.float8e4
AX = mybir.AxisListType
ALU = mybir.AluOpType
AF = mybir.ActivationFunctionType
PM = mybir.MatmulPerfMode


def legalize_multi_waits(nc):
    """This walrus build supports only one sync-wait per instruction; hoist
    extra waits onto same-engine NoOps placed immediately before."""
    nid = 0
    for fn in nc.m.functions:
        for bb in fn.blocks:
            new = []
            changed = False
            for inst in bb.instructions:
                si = inst.sync_info
                waits = si.on_wait if si is not None else []
                if len(waits) > 1:
                    for w in waits[:-1]:
                        nop = mybir.InstNoOp(name=f"I-waitfix-{nid}")
                        nid += 1
                        nop.engine = inst.engine
                        nop.sync_info = mybir.SyncInfo(on_wait=[w], on_update=[])
                        new.append(nop)
                    si.on_wait = [waits[-1]]
                    inst.sync_info = si
                    changed = True
                new.append(inst)
            if changed:
                bb.instructions = new


def build_nc(use_mask=True):
    nc = bass.Bass()

    xt_ext = nc.declare_dram_parameter("xt", [P, KT * N], bf16, isOutput=False)
    wqks_ext = nc.declare_dram_parameter("wqks", [P, KT * 4 * D], bf16,
                                         isOutput=False)
    wv_ext = nc.declare_dram_parameter("wv", [P, KT * D], bf16, isOutput=False)
    biasc_ext = nc.declare_dram_parameter("biasc", [D, 5], f32, isOutput=False)
    maskrow_ext = nc.declare_dram_parameter("maskrow", [1, N], f32,
                                            isOutput=False)
    wo_ext = nc.declare_dram_parameter("wo", [D, DIM], bf16, isOutput=False)
    cs_ext = nc.declare_dram_parameter("cs", [D, 2 * N], bf16, isOutput=False)
    tconst_ext = nc.declare_dram_parameter("tconst", [D, NG * P], fp8,
                                           isOutput=False)
    m512_ext = nc.declare_dram_parameter("m512", [P, N], bf16, isOutput=False)
    identf_ext = nc.declare_dram_parameter("identf", [P, P], f32, isOutput=False)
    # rel stream: [tile, p=(i_l*16+d_l), (block16, g4, j)] fp8; 1MB pieces
    rel_ext = nc.declare_dram_parameter("rel", [NT, P, NBT * NG * N], fp8,
                                        isOutput=False)
    out_ext = nc.declare_dram_parameter("out", [N, DIM], bf16, isOutput=True)

    with tile.TileContext(nc) as tc, ExitStack() as ctx:
        dma = nc.sync      # HWDGE queue 1: the whole rel stream
        dma2 = nc.scalar   # HWDGE queue 2: inputs first, then outputs
        consts = ctx.enter_context(tc.tile_pool(name="consts", bufs=1))
        pro = ctx.enter_context(tc.tile_pool(name="pro", bufs=1))
        relp = ctx.enter_context(tc.tile_pool(name="relp", bufs=1))
        smp = ctx.enter_context(tc.tile_pool(name="smp", bufs=2))
        smallp = ctx.enter_context(tc.tile_pool(name="smallp", bufs=2))
        outp = ctx.enter_context(tc.tile_pool(name="outp", bufs=2))
        op = ctx.enter_context(tc.tile_pool(name="op", bufs=1))
        psA = ctx.enter_context(
            tc.tile_pool(name="psA", bufs=2, space=bass.MemorySpace.PSUM))
        psB = ctx.enter_context(
            tc.tile_pool(name="psB", bufs=3, space=bass.MemorySpace.PSUM))
        psW = ctx.enter_context(
            tc.tile_pool(name="psW", bufs=1, space=bass.MemorySpace.PSUM))
        psV = ctx.enter_context(
            tc.tile_pool(name="psV", bufs=1, space=bass.MemorySpace.PSUM))
        psO = ctx.enter_context(
            tc.tile_pool(name="psO", bufs=1, space=bass.MemorySpace.PSUM))

        # ---- qpad zero-fill FIRST: f32-bitcast memsets split across
        # gpsimd and DVE (both otherwise idle at t0) ----
        qpad = consts.tile([P, NGP, 2, NT, PW], fp8, name="qpad")
        half0 = qpad[:, 0:1].rearrange("p a t u w -> p (a t u w)").bitcast(f32)
        half1 = qpad[:, 1:2].rearrange("p a t u w -> p (a t u w)").bitcast(f32)
        nc.gpsimd.memset(half0, 0.0)
        nc.vector.memset(half1, 0.0)

        # ---- inputs on queue 2 (scalar), emitted before any ACT work ----
        xt_sb = pro.tile([P, KT, N], bf16)
        dma2.dma_start(out=xt_sb[:], in_=xt_ext.rearrange("p (u n) -> p u n", u=KT))
        wqks_sb = pro.tile([P, KT, 4 * D], bf16)
        dma2.dma_start(out=wqks_sb[:],
                       in_=wqks_ext.rearrange("p (u m) -> p u m", u=KT))
        wv_sb = pro.tile([P, KT, D], bf16)
        dma2.dma_start(out=wv_sb[:], in_=wv_ext.rearrange("p (u m) -> p u m", u=KT))
        bias_cols = consts.tile([D, 5], f32)
        dma2.dma_start(out=bias_cols[:], in_=biasc_ext[:])
        maskrow_sb = consts.tile([1, N], f32)
        if use_mask:
            dma2.dma_start(out=maskrow_sb[:], in_=maskrow_ext[:])
        cs_sb = consts.tile([D, 2, N], bf16)
        dma2.dma_start(out=cs_sb[:], in_=cs_ext.rearrange("d (c n) -> d c n", c=2))
        tconst_sb = consts.tile([D, NG, P], fp8)
        dma2.dma_start(out=tconst_sb[:],
                       in_=tconst_ext.rearrange("d (g p) -> d g p", g=NG))
        m512_sb = consts.tile([P, N], bf16)
        dma2.dma_start(out=m512_sb[:], in_=m512_ext[:])
        identf = consts.tile([P, P], f32)
        dma2.dma_start(out=identf[:], in_=identf_ext[:])
        wo_sb = consts.tile([D, DIM], bf16)
        dma2.dma_start(out=wo_sb[:], in_=wo_ext[:])
        ones_sb = consts.tile([1, N], f32)
        nc.vector.memset(ones_sb, 1.0)

        # ---- rel stream: 16 x 1MB pieces, all on the sync queue; the
        # final piece split into two 512KB halves for a short PE tail ----
        rel_tiles = {}
        psz = NBT * NG * N // NT   # free elems per piece (4 blocks)
        for it in range(NT):
            for pc in range(NT):
                if (it, pc) == (NT - 1, NT - 1):
                    rpa = relp.tile([P, psz // 2], fp8, name="rel15a")
                    dma.dma_start(out=rpa[:],
                                  in_=rel_ext[it, :, pc * psz:pc * psz + psz // 2])
                    rpb = relp.tile([P, psz // 2], fp8, name="rel15b")
                    dma.dma_start(out=rpb[:],
                                  in_=rel_ext[it, :, pc * psz + psz // 2:(pc + 1) * psz])
                    rel_tiles[(it, pc)] = (rpa, rpb)
                else:
                    rp = relp.tile([P, psz], fp8, name=f"rel{it}_{pc}")
                    dma.dma_start(out=rp[:],
                                  in_=rel_ext[it, :, pc * psz:(pc + 1) * psz])
                    rel_tiles[(it, pc)] = rp

        def rel_mv(it, bl, gp):
            """moving AP [128, 2, 512] for (tile, block16, g-pair)"""
            pc, bi = divmod(bl, NBT // NT)   # piece, block-in-piece
            rp = rel_tiles[(it, pc)]
            if isinstance(rp, tuple):
                # halves split blocks 12,13 / 14,15 of tile 3
                rp = rp[0] if bi < 2 else rp[1]
                bi = bi % 2
            off = (bi * NG + 2 * gp) * N
            full = rp[:]
            return AP(full.tensor, full.offset + off,
                      [list(full.ap[0]), [N, 2], [1, N]])

        # ---- projections (transposed): qT, kT, qrotT, krotT, vT ----
        qkT_sb = pro.tile([D, 4, N], bf16)
        for c in range(4):
            ps_c = psA.tile([P, N], f32, tag="big")
            for u in range(KT):
                nc.tensor.matmul(ps_c[0:D, :],
                                 wqks_sb[:, u, c * D:(c + 1) * D],
                                 xt_sb[:, u, :], start=(u == 0), stop=(u == KT - 1))
            nc.scalar.activation(qkT_sb[:, c, :], ps_c[0:D, :], AF.Identity,
                                 bias=bias_cols[:, c:c + 1],
                                 scale=(KF if c in (1, 3) else 1.0))

        ps_v = psA.tile([P, N], f32, tag="big")
        for u in range(KT):
            nc.tensor.matmul(ps_v[0:D, :], wv_sb[:, u, :], xt_sb[:, u, :],
                             start=(u == 0), stop=(u == KT - 1))
        vT_sb = pro.tile([D, N], f32)
        nc.scalar.activation(vT_sb[:], ps_v[0:D, :], AF.Identity,
                             bias=bias_cols[:, 4:5])

        # ---- RoPE on DVE ----
        qkp_sb = pro.tile([D, 2, N], bf16)
        t1 = pro.tile([D, N], bf16, tag="ropet1")
        t2 = pro.tile([D, N], bf16, tag="ropet2")
        for c in range(2):  # 0: q, 1: k
            nc.vector.tensor_mul(t1[:], qkT_sb[:, c, :], cs_sb[:, 0, :])
            nc.vector.tensor_mul(t2[:], qkT_sb[:, 2 + c, :], cs_sb[:, 1, :])
            nc.vector.tensor_add(qkp_sb[:, c, :], t1[:], t2[:])
        qpT = qkp_sb[:, 0, :]
        kpT = qkp_sb[:, 1, :]

        # ---- v -> [j, d] layout via PE transposes ----
        v_sb = pro.tile([P, NT, D], bf16)
        for jt in range(NT):
            pv = psW.tile([P, P], f32, tag="tp")
            nc.tensor.transpose(pv[:, 0:D], vT_sb[:, jt * P:(jt + 1) * P],
                                identf[0:D, 0:D])
            nc.scalar.copy(v_sb[:, jt, :], pv[:, 0:D])

        # ---- Qpad strips: Rep_g[p, n] = q'T[g*16 + p%16, n]; strips go to
        # plane (gp=g//2, t=g%2): [u, 128*bl + c] (c = p//16 in-strip col),
        # read back as windows [u, 120*bl : 120*bl+128]. ----
        qpad_full = qpad[:]
        ppair = list(qpad_full.ap[0])

        def emit_strip(g):
            ps_rep = psB.tile([P, N], f32, tag="rep")
            nc.tensor.matmul(ps_rep[:], tconst_sb[:, g, :], qpT,
                             start=True, stop=True)
            strip_out = AP(qpad_full.tensor,
                           qpad_full.offset + g * (NT * PW),
                           [ppair, [PW, NT], [P, NBT], [1, IB]])
            nc.vector.tensor_mul(
                strip_out,
                ps_rep.rearrange("p (u v c) -> p u v c", u=NT, v=NBT)[:],
                m512_sb.rearrange("p (u v c) -> p u v c", u=NT, v=NBT)[:])

        # ---- main loop; tile 0 interleaves strip construction ----
        o_tiles = [op.tile([P, DIM], bf16, name=f"o{it}") for it in range(NT)]
        for it in range(NT):
            dots_ps = psA.tile([P, N], f32, tag="big")
            nc.tensor.matmul(dots_ps[:], qpT[:, it * P:(it + 1) * P], kpT,
                             start=True, stop=False, skip_group_check=True)
            if use_mask:
                nc.tensor.matmul(dots_ps[:], ones_sb[:, 0:P], maskrow_sb[:],
                                 start=False, stop=False, skip_group_check=True)
            if it == 0:
                # gp-outer: build strips for pair gp, then sweep its blocks
                for gp in range(NGP):
                    emit_strip(2 * gp)
                    emit_strip(2 * gp + 1)
                    for bl in range(NBT):
                        nc.tensor.matmul(
                            dots_ps[:],
                            qpad[:, gp, :, it, WS * bl:WS * bl + P],
                            rel_mv(it, bl, gp),
                            start=False,
                            stop=(gp == NGP - 1 and bl == NBT - 1),
                            perf_mode=PM.DoubleRow,
                            skip_group_check=True)
            else:
                for bl in range(NBT):
                    for gp in range(NGP):
                        nc.tensor.matmul(
                            dots_ps[:],
                            qpad[:, gp, :, it, WS * bl:WS * bl + P],
                            rel_mv(it, bl, gp),
                            start=False,
                            stop=(bl == NBT - 1 and gp == NGP - 1),
                            perf_mode=PM.DoubleRow,
                            skip_group_check=True)

            # softmax: unnormalized exp(dots/RELSCALE), no max-subtraction;
            # 4 column chunks so transposes/attnV pipeline behind exp
            w_sm = smp.tile([P, N], f32, tag="w_sm")
            rowsum4 = smallp.tile([P, NT], f32, tag="rowsum4")
            wT_sb = outp.tile([P, NT, P], bf16, tag="wT_sb")
            attn_ps = psV.tile([D, P], f32, tag="attn")
            for jt in range(NT):
                nc.scalar.activation(w_sm[:, jt * P:(jt + 1) * P],
                                     dots_ps[:, jt * P:(jt + 1) * P], AF.Exp,
                                     scale=1.0 / RELSCALE,
                                     accum_out=rowsum4[:, jt:jt + 1])
                wp = psW.tile([P, P], f32, tag="tp")
                nc.tensor.transpose(wp[:], w_sm[:, jt * P:(jt + 1) * P], identf[:])
                nc.vector.tensor_copy(wT_sb[:, jt, :], wp[:])
                nc.tensor.matmul(attn_ps[:], v_sb[:, jt, :], wT_sb[:, jt, :],
                                 start=(jt == 0), stop=(jt == NT - 1))
            rowsum = smallp.tile([P, 1], f32, tag="rowsum")
            nc.vector.tensor_reduce(rowsum[:], rowsum4[:], AX.X, ALU.add)
            rcp = smallp.tile([P, 1], f32, tag="rcp")
            nc.vector.reciprocal(rcp[:], rowsum[:])

            attn_sb = outp.tile([D, P], bf16, tag="attn_sb")
            nc.scalar.copy(attn_sb[:], attn_ps[:])
            out_ps = psO.tile([P, DIM], f32, tag="out")
            nc.tensor.matmul(out_ps[:], attn_sb[:], wo_sb[:], start=True, stop=True)
            nc.scalar.activation(o_tiles[it][:], out_ps[:], AF.Copy, scale=rcp[:])
            dma2.dma_start(out=out_ext[it * P:(it + 1) * P, :], in_=o_tiles[it][:])

    legalize_multi_waits(nc)
    return nc


_NC_CACHE = None
TRACE = False
LAST_RESULT = None


def _get_nc(use_mask):
    global _NC_CACHE
    if _NC_CACHE is None or _NC_CACHE[1] != use_mask:
        _NC_CACHE = (build_nc(use_mask), use_mask)
    return _NC_CACHE[0]


def _rot_mat():
    """rotate_half as a right-multiply matrix: rot(q) = q @ Rm."""
    Rm = np.zeros((D, D), np.float32)
    for i in range(D // 2):
        Rm[2 * i + 1, 2 * i] = -1.0
        Rm[2 * i, 2 * i + 1] = 1.0
    return Rm


def kernel(**inputs):
    x = np.asarray(inputs["x"], dtype=np.float32)
    mask = np.asarray(inputs["mask"])
    rope = np.asarray(inputs["rope"], dtype=np.float32)
    rel_pos = np.asarray(inputs["rel_pos"], dtype=np.float32)
    Wq = np.asarray(inputs["Wq"], dtype=np.float32)
    bq = np.asarray(inputs["bq"], dtype=np.float32)
    Wk = np.asarray(inputs["Wk"], dtype=np.float32)
    bk = np.asarray(inputs["bk"], dtype=np.float32)
    Wv = np.asarray(inputs["Wv"], dtype=np.float32)
    bv = np.asarray(inputs["bv"], dtype=np.float32)
    Wo = np.asarray(inputs["Wo"], dtype=np.float32)
    bo = np.asarray(inputs["bo"], dtype=np.float32)

    use_mask = not bool(np.asarray(mask).all())
    nc = _get_nc(use_mask)
    Rm = _rot_mat()

    def swz(a):  # [K, M] -> [p, (u, M)] with K = (u, p)
        k, m = a.shape
        return np.ascontiguousarray(
            a.reshape(KT, P, m).transpose(1, 0, 2).reshape(P, KT * m))

    xT = swz(x.reshape(N, DIM).T.astype(np.float32)).astype(ml_dtypes.bfloat16)
    maskrow = ((mask.reshape(1, N).astype(np.float32)) - 1.0) * NEG_BIG

    cosT = np.cos(rope).T.astype(np.float32)
    sinT = np.sin(rope).T.astype(np.float32)
    cs = np.concatenate([cosT, sinT], axis=1).astype(ml_dtypes.bfloat16)

    # T[d, g, p] = (d == g*DG + p%DG); m512[p, n] = (n%IB == p//DG)
    d_i = np.arange(D)[:, None, None]
    g_i = np.arange(NG)[None, :, None]
    p_i = np.arange(P)[None, None, :]
    tconst = (d_i == g_i * DG + p_i % DG).astype(np.float32)
    tconst = tconst.reshape(D, NG * P).astype(ml_dtypes.float8_e4m3)
    p_2 = np.arange(P)[:, None]
    n_2 = np.arange(N)[None, :]
    m512 = ((n_2 % IB) == (p_2 // DG)).astype(np.float32)
    m512 = m512.astype(ml_dtypes.bfloat16)

    identf = np.eye(P, dtype=np.float32)

    # rel: [h, it, p=(i_l*DG+d_l), (block16, g, j)] fp8, scaled by RELSCALE
    rel8 = (rel_pos[0] * RELSCALE).astype(ml_dtypes.float8_e4m3)
    # [h, (it, b16, i_l), j, (g, d_l)] -> [h, it, i_l, d_l, b16, g, j]
    rel8 = rel8.reshape(H, NT, NBT, IB, N, NG, DG)
    rel8 = np.ascontiguousarray(rel8.transpose(0, 1, 3, 6, 2, 5, 4))
    rel8 = rel8.reshape(H, NT, P, NBT * NG * N)

    in_maps = []
    for h in range(N_CORES):
        sl = slice(h * D, (h + 1) * D)
        wq, wk = Wq[:, sl], Wk[:, sl]
        wqks = np.concatenate([wq, wk, wq @ Rm, wk @ Rm], axis=1)
        biasc = np.stack([bq[sl], bk[sl] * KF, bq[sl] @ Rm,
                          (bk[sl] @ Rm) * KF, bv[sl]], axis=1).astype(np.float32)
        in_maps.append({
            "xt": xT,
            "wqks": swz(wqks).astype(ml_dtypes.bfloat16),
            "wv": swz(np.ascontiguousarray(Wv[:, sl])).astype(ml_dtypes.bfloat16),
            "biasc": np.ascontiguousarray(biasc),
            "maskrow": np.ascontiguousarray(maskrow),
            "wo": np.ascontiguousarray(Wo[sl, :]).astype(ml_dtypes.bfloat16),
            "cs": cs,
            "tconst": tconst,
            "m512": m512,
            "identf": identf,
            "rel": rel8[h],
        })

    from concourse.bass_utils import run_bass_kernel_spmd
    res = run_bass_kernel_spmd(nc, in_maps, list(range(N_CORES)), trace=TRACE)
    globals()["LAST_RESULT"] = res
    out = np.zeros((N, DIM), dtype=np.float32)
    for h in range(N_CORES):
        out += np.asarray(res.results[h]["out"], dtype=np.float32)
    out += bo[None, :]
    return out.reshape(B, N, DIM)


# revision 9
# speedup vs baseline: 1.2964x; 1.0457x over previous
"""Bass/Trainium2 kernel for nn_Attention_66297115181568 (sparse_attention).

Strategy: head-parallel across 8 NeuronCores; core h computes head h
end-to-end and its 64-row slice of the Wo projection. The host sums the
8 partial (512, 512) outputs (the tensor-parallel all-reduce) and adds bo.

v4 (lessons from v1=76us, v2=102us, v3=90us):
  1. rel-term matmuls in fp8 DoubleRow perf mode (measured 216ns per
     [128, 2, 512] matmul at cruise = 2 fp8/partition/cycle, 2x the
     bf16 col-tiled scheme). DoubleRow is ISA-incompatible with PE
     column tiling, so each (block, g-pair) stationary is a full
     128-column [128, 2, 128] window, zero outside its block strip.
  2. IB=8 block geometry (partitions = 8 rows x 16 d): only NG=4
     d-groups -> 4 repl matmuls + 4 strip-muls (v3 had 16+16, a ~13us
     serial prologue chain). Windows for the 16 blocks of a tile are
     overlapped in a [1928]-elem plane per (gp, t, u): window bl starts
     at 120*bl, strip bl at 128*bl, so each window holds exactly its
     own strip (at column 8*bl+c) and shared zeros elsewhere.
  3. qpad zero-fill as f32-bitcast memsets (4B/cycle instead of 1),
     split gpsimd/DVE, emitted first.
  4. DMA: ALL 16 1MB rel pieces on the sync queue (423 GB/s measured;
     v3's split put pieces behind main-loop ACT doorbells -> 87us
     stragglers). Inputs on the scalar queue, emitted first, so the rel
     stream starts at t~9us (queue-up latency) with no input prefix.
     Outputs (bf16) also on the scalar queue.
  5. tile 0 consumes g-pairs as they are built (repl/strip/matmul
     interleave) instead of waiting for all strips.
  6. Softmax: exp(dots/64) with no max-subtraction; the x64 logit scale
     rides in the k-projection activation (KF); exp in 4 column chunks
     so w-transposes/attnV pipeline behind it.
"""

import sys

sys.path.insert(0, "/opt/trn_rl_repo")

from contextlib import ExitStack

import numpy as np
import ml_dtypes

import concourse.bass as bass
import concourse.tile as tile
from concourse import mybir
from concourse.ap import AP

# problem dims (hardcoded per spec)
B, N, DIM, H, D = 1, 512, 512, 8, 64
INNER = H * D
N_CORES = 8
P = 128                 # SBUF partitions
NT = N // P             # 4 row tiles
KT = DIM // P           # 4 contraction tiles for projections
IB = 8                  # i-block rows per partition-group
NBT = P // IB           # 16 blocks per row tile
DG = P // IB            # 16 d's per d-group
NG = D // DG            # 4 d-groups
NGP = NG // 2           # 2 d-group pairs (DoubleRow)
WS = P - IB             # 120: window stride in the packed plane
PW = WS * (NBT - 1) + P  # 1928: packed plane width
SCALE = D ** -0.5
NEG_BIG = 1.0e36
RELSCALE = 64.0         # host scales rel by this before fp8 cast
KF = SCALE * RELSCALE   # folded into the k/krot projection activation

f32 = mybir.dt.float32
bf16 = mybir.dt.bfloat16
fp8 = mybir.dt.float8e4
AX = mybir.AxisListType
ALU = mybir.AluOpType
AF = mybir.ActivationFunctionType
PM = mybir.MatmulPerfMode


def legalize_multi_waits(nc):
    """This walrus build supports only one sync-wait per instruction; hoist
    extra waits onto same-engine NoOps placed immediately before."""
    nid = 0
    for fn in nc.m.functions:
        for bb in fn.blocks:
            new = []
            changed = False
            for inst in bb.instructions:
                si = inst.sync_info
                waits = si.on_wait if si is not None else []
                if len(waits) > 1:
                    for w in waits[:-1]:
                        nop = mybir.InstNoOp(name=f"I-waitfix-{nid}")
                        nid += 1
                        nop.engine = inst.engine
                        nop.sync_info = mybir.SyncInfo(on_wait=[w], on_update=[])
                        new.append(nop)
                    si.on_wait = [waits[-1]]
                    inst.sync_info = si
                    changed = True
                new.append(inst)
            if changed:
                bb.instructions = new


def build_nc(use_mask=True):
    nc = bass.Bass()

    xt_ext = nc.declare_dram_parameter("xt", [P, KT * N], bf16, isOutput=False)
    wqks_ext = nc.declare_dram_parameter("wqks", [P, KT * 4 * D], bf16,
                                         isOutput=False)
    wv_ext = nc.declare_dram_parameter("wv", [P, KT * D], bf16, isOutput=False)
    biasc_ext = nc.declare_dram_parameter("biasc", [D, 5], f32, isOutput=False)
    maskrow_ext = nc.declare_dram_parameter("maskrow", [1, N], f32,
                                            isOutput=False)
    wo_ext = nc.declare_dram_parameter("wo", [D, DIM], bf16, isOutput=False)
    cs_ext = nc.declare_dram_parameter("cs", [D, 2 * N], bf16, isOutput=False)
    tconst_ext = nc.declare_dram_parameter("tconst", [D, NG * P], fp8,
                                           isOutput=False)
    m512_ext = nc.declare_dram_parameter("m512", [P, N], bf16, isOutput=False)
    identf_ext = nc.declare_dram_parameter("identf", [P, P], f32, isOutput=False)
    # rel stream: [tile, p=(i_l*16+d_l), (block16, g4, j)] fp8; 1MB pieces
    rel_ext = nc.declare_dram_parameter("rel", [NT, P, NBT * NG * N], fp8,
                                        isOutput=False)
    out_ext = nc.declare_dram_parameter("out", [N, DIM], bf16, isOutput=True)

    with tile.TileContext(nc) as tc, ExitStack() as ctx:
        dma = nc.sync      # HWDGE queue 1: the whole rel stream
        dma2 = nc.scalar   # HWDGE queue 2: inputs first, then outputs
        consts = ctx.enter_context(tc.tile_pool(name="consts", bufs=1))
        pro = ctx.enter_context(tc.tile_pool(name="pro", bufs=1))
        relp = ctx.enter_context(tc.tile_pool(name="relp", bufs=1))
        smp = ctx.enter_context(tc.tile_pool(name="smp", bufs=2))
        smallp = ctx.enter_context(tc.tile_pool(name="smallp", bufs=2))
        outp = ctx.enter_context(tc.tile_pool(name="outp", bufs=2))
        op = ctx.enter_context(tc.tile_pool(name="op", bufs=1))
        psA = ctx.enter_context(
            tc.tile_pool(name="psA", bufs=2, space=bass.MemorySpace.PSUM))
        psB = ctx.enter_context(
            tc.tile_pool(name="psB", bufs=3, space=bass.MemorySpace.PSUM))
        psW = ctx.enter_context(
            tc.tile_pool(name="psW", bufs=1, space=bass.MemorySpace.PSUM))
        psV = ctx.enter_context(
            tc.tile_pool(name="psV", bufs=1, space=bass.MemorySpace.PSUM))
        psO = ctx.enter_context(
            tc.tile_pool(name="psO", bufs=1, space=bass.MemorySpace.PSUM))

        # ---- qpad zero-fill FIRST: f32-bitcast memsets split across
        # gpsimd and DVE (both otherwise idle at t0) ----
        qpad = consts.tile([P, NGP, 2, NT, PW], fp8, name="qpad")
        half0 = qpad[:, 0:1].rearrange("p a t u w -> p (a t u w)").bitcast(f32)
        half1 = qpad[:, 1:2].rearrange("p a t u w -> p (a t u w)").bitcast(f32)
        nc.gpsimd.memset(half0, 0.0)
        nc.vector.memset(half1, 0.0)

        # ---- inputs on the sync queue, ahead of the rel stream (v1-proven
        # arrangement: one queue, inputs as the FIFO prefix; the scalar
        # queue carries only outputs so main-loop ACT work never delays a
        # transfer doorbell) ----
        xt_sb = pro.tile([P, KT, N], bf16)
        dma.dma_start(out=xt_sb[:], in_=xt_ext.rearrange("p (u n) -> p u n", u=KT))
        wqks_sb = pro.tile([P, KT, 4 * D], bf16)
        dma.dma_start(out=wqks_sb[:],
                      in_=wqks_ext.rearrange("p (u m) -> p u m", u=KT))
        wv_sb = pro.tile([P, KT, D], bf16)
        dma.dma_start(out=wv_sb[:], in_=wv_ext.rearrange("p (u m) -> p u m", u=KT))
        bias_cols = consts.tile([D, 5], f32)
        dma.dma_start(out=bias_cols[:], in_=biasc_ext[:])
        maskrow_sb = consts.tile([1, N], f32)
        if use_mask:
            dma.dma_start(out=maskrow_sb[:], in_=maskrow_ext[:])
        cs_sb = consts.tile([D, 2, N], bf16)
        dma.dma_start(out=cs_sb[:], in_=cs_ext.rearrange("d (c n) -> d c n", c=2))
        tconst_sb = consts.tile([D, NG, P], fp8)
        dma.dma_start(out=tconst_sb[:],
                      in_=tconst_ext.rearrange("d (g p) -> d g p", g=NG))
        m512_sb = consts.tile([P, N], bf16)
        dma.dma_start(out=m512_sb[:], in_=m512_ext[:])
        identf = consts.tile([P, P], f32)
        dma.dma_start(out=identf[:], in_=identf_ext[:])
        wo_sb = consts.tile([D, DIM], bf16)
        dma.dma_start(out=wo_sb[:], in_=wo_ext[:])
        ones_sb = consts.tile([1, N], f32)
        nc.vector.memset(ones_sb, 1.0)

        # ---- rel stream: 2MB pieces (2 per row tile), all on the sync
        # queue; the final piece split into two 1MB halves for a short
        # PE tail ----
        rel_tiles = {}
        hpt = NBT * NG * N // 2    # free elems per half-tile piece (8 blocks)
        for it in range(NT):
            for pc in range(2):
                if (it, pc) == (NT - 1, 1):
                    rpa = relp.tile([P, hpt // 2], fp8, name="rel7a")
                    dma.dma_start(out=rpa[:],
                                  in_=rel_ext[it, :, pc * hpt:pc * hpt + hpt // 2])
                    rpb = relp.tile([P, hpt // 2], fp8, name="rel7b")
                    dma.dma_start(out=rpb[:],
                                  in_=rel_ext[it, :, pc * hpt + hpt // 2:(pc + 1) * hpt])
                    rel_tiles[(it, pc)] = (rpa, rpb)
                else:
                    rp = relp.tile([P, hpt], fp8, name=f"rel{it}_{pc}")
                    dma.dma_start(out=rp[:],
                                  in_=rel_ext[it, :, pc * hpt:(pc + 1) * hpt])
                    rel_tiles[(it, pc)] = rp

        def rel_mv(it, bl, gp):
            """moving AP [128, 2, 512] for (tile, block16, g-pair)"""
            pc, bi = divmod(bl, NBT // 2)   # half-tile piece, block-in-piece
            rp = rel_tiles[(it, pc)]
            if isinstance(rp, tuple):
                # halves split blocks 8-11 / 12-15 of tile 3
                rp = rp[0] if bi < 4 else rp[1]
                bi = bi % 4
            off = (bi * NG + 2 * gp) * N
            full = rp[:]
            return AP(full.tensor, full.offset + off,
                      [list(full.ap[0]), [N, 2], [1, N]])

        # ---- projections (transposed): qT, kT, qrotT, krotT, vT ----
        qkT_sb = pro.tile([D, 4, N], bf16)
        for c in range(4):
            ps_c = psA.tile([P, N], f32, tag="big")
            for u in range(KT):
                nc.tensor.matmul(ps_c[0:D, :],
                                 wqks_sb[:, u, c * D:(c + 1) * D],
                                 xt_sb[:, u, :], start=(u == 0), stop=(u == KT - 1))
            nc.scalar.activation(qkT_sb[:, c, :], ps_c[0:D, :], AF.Identity,
                                 bias=bias_cols[:, c:c + 1],
                                 scale=(KF if c in (1, 3) else 1.0))

        ps_v = psA.tile([P, N], f32, tag="big")
        for u in range(KT):
            nc.tensor.matmul(ps_v[0:D, :], wv_sb[:, u, :], xt_sb[:, u, :],
                             start=(u == 0), stop=(u == KT - 1))
        vT_sb = pro.tile([D, N], f32)
        nc.scalar.activation(vT_sb[:], ps_v[0:D, :], AF.Identity,
                             bias=bias_cols[:, 4:5])

        # ---- RoPE on DVE ----
        qkp_sb = pro.tile([D, 2, N], bf16)
        t1 = pro.tile([D, N], bf16, tag="ropet1")
        t2 = pro.tile([D, N], bf16, tag="ropet2")
        for c in range(2):  # 0: q, 1: k
            nc.vector.tensor_mul(t1[:], qkT_sb[:, c, :], cs_sb[:, 0, :])
            nc.vector.tensor_mul(t2[:], qkT_sb[:, 2 + c, :], cs_sb[:, 1, :])
            nc.vector.tensor_add(qkp_sb[:, c, :], t1[:], t2[:])
        qpT = qkp_sb[:, 0, :]
        kpT = qkp_sb[:, 1, :]

        # ---- v -> [j, d] layout via PE transposes ----
        v_sb = pro.tile([P, NT, D], bf16)
        for jt in range(NT):
            pv = psW.tile([P, P], f32, tag="tp")
            nc.tensor.transpose(pv[:, 0:D], vT_sb[:, jt * P:(jt + 1) * P],
                                identf[0:D, 0:D])
            nc.scalar.copy(v_sb[:, jt, :], pv[:, 0:D])

        # ---- Qpad strips: Rep_g[p, n] = q'T[g*16 + p%16, n]; strips go to
        # plane (gp=g//2, t=g%2): [u, 128*bl + c] (c = p//16 in-strip col),
        # read back as windows [u, 120*bl : 120*bl+128]. ----
        qpad_full = qpad[:]
        ppair = list(qpad_full.ap[0])

        def emit_strip(g):
            ps_rep = psB.tile([P, N], f32, tag="rep")
            nc.tensor.matmul(ps_rep[:], tconst_sb[:, g, :], qpT,
                             start=True, stop=True)
            strip_out = AP(qpad_full.tensor,
                           qpad_full.offset + g * (NT * PW),
                           [ppair, [PW, NT], [P, NBT], [1, IB]])
            nc.vector.tensor_mul(
                strip_out,
                ps_rep.rearrange("p (u v c) -> p u v c", u=NT, v=NBT)[:],
                m512_sb.rearrange("p (u v c) -> p u v c", u=NT, v=NBT)[:])

        # ---- main loop; tile 0 interleaves strip construction ----
        o_tiles = [op.tile([P, DIM], bf16, name=f"o{it}") for it in range(NT)]
        for it in range(NT):
            dots_ps = psA.tile([P, N], f32, tag="big")
            nc.tensor.matmul(dots_ps[:], qpT[:, it * P:(it + 1) * P], kpT,
                             start=True, stop=False, skip_group_check=True)
            if use_mask:
                nc.tensor.matmul(dots_ps[:], ones_sb[:, 0:P], maskrow_sb[:],
                                 start=False, stop=False, skip_group_check=True)
            if it == 0:
                # gp-outer: build strips for pair gp, then sweep its blocks
                for gp in range(NGP):
                    emit_strip(2 * gp)
                    emit_strip(2 * gp + 1)
                    for bl in range(NBT):
                        nc.tensor.matmul(
                            dots_ps[:],
                            qpad[:, gp, :, it, WS * bl:WS * bl + P],
                            rel_mv(it, bl, gp),
                            start=False,
                            stop=(gp == NGP - 1 and bl == NBT - 1),
                            perf_mode=PM.DoubleRow,
                            skip_group_check=True)
            else:
                for bl in range(NBT):
                    for gp in range(NGP):
                        nc.tensor.matmul(
                            dots_ps[:],
                            qpad[:, gp, :, it, WS * bl:WS * bl + P],
                            rel_mv(it, bl, gp),
                            start=False,
                            stop=(bl == NBT - 1 and gp == NGP - 1),
                            perf_mode=PM.DoubleRow,
                            skip_group_check=True)

            # softmax: unnormalized exp(dots/RELSCALE), no max-subtraction;
            # 4 column chunks so transposes/attnV pipeline behind exp
            w_sm = smp.tile([P, N], f32, tag="w_sm")
            rowsum4 = smallp.tile([P, NT], f32, tag="rowsum4")
            wT_sb = outp.tile([P, NT, P], bf16, tag="wT_sb")
            attn_ps = psV.tile([D, P], f32, tag="attn")
            for jt in range(NT):
                nc.scalar.activation(w_sm[:, jt * P:(jt + 1) * P],
                                     dots_ps[:, jt * P:(jt + 1) * P], AF.Exp,
                                     scale=1.0 / RELSCALE,
                                     accum_out=rowsum4[:, jt:jt + 1])
                wp = psW.tile([P, P], f32, tag="tp")
                nc.tensor.transpose(wp[:], w_sm[:, jt * P:(jt + 1) * P], identf[:])
                nc.vector.tensor_copy(wT_sb[:, jt, :], wp[:])
                nc.tensor.matmul(attn_ps[:], v_sb[:, jt, :], wT_sb[:, jt, :],
                                 start=(jt == 0), stop=(jt == NT - 1))
            rowsum = smallp.tile([P, 1], f32, tag="rowsum")
            nc.vector.tensor_reduce(rowsum[:], rowsum4[:], AX.X, ALU.add)
            rcp = smallp.tile([P, 1], f32, tag="rcp")
            nc.vector.reciprocal(rcp[:], rowsum[:])

            attn_sb = outp.tile([D, P], bf16, tag="attn_sb")
            nc.scalar.copy(attn_sb[:], attn_ps[:])
            out_ps = psO.tile([P, DIM], f32, tag="out")
            nc.tensor.matmul(out_ps[:], attn_sb[:], wo_sb[:], start=True, stop=True)
            nc.scalar.activation(o_tiles[it][:], out_ps[:], AF.Copy, scale=rcp[:])
            dma2.dma_start(out=out_ext[it * P:(it + 1) * P, :], in_=o_tiles[it][:])

    legalize_multi_waits(nc)
    return nc


_NC_CACHE = None
TRACE = False
LAST_RESULT = None


def _get_nc(use_mask):
    global _NC_CACHE
    if _NC_CACHE is None or _NC_CACHE[1] != use_mask:
        _NC_CACHE = (build_nc(use_mask), use_mask)
    return _NC_CACHE[0]


def _rot_mat():
    """rotate_half as a right-multiply matrix: rot(q) = q @ Rm."""
    Rm = np.zeros((D, D), np.float32)
    for i in range(D // 2):
        Rm[2 * i + 1, 2 * i] = -1.0
        Rm[2 * i, 2 * i + 1] = 1.0
    return Rm


def kernel(**inputs):
    x = np.asarray(inputs["x"], dtype=np.float32)
    mask = np.asarray(inputs["mask"])
    rope = np.asarray(inputs["rope"], dtype=np.float32)
    rel_pos = np.asarray(inputs["rel_pos"], dtype=np.float32)
    Wq = np.asarray(inputs["Wq"], dtype=np.float32)
    bq = np.asarray(inputs["bq"], dtype=np.float32)
    Wk = np.asarray(inputs["Wk"], dtype=np.float32)
    bk = np.asarray(inputs["bk"], dtype=np.float32)
    Wv = np.asarray(inputs["Wv"], dtype=np.float32)
    bv = np.asarray(inputs["bv"], dtype=np.float32)
    Wo = np.asarray(inputs["Wo"], dtype=np.float32)
    bo = np.asarray(inputs["bo"], dtype=np.float32)

    use_mask = not bool(np.asarray(mask).all())
    nc = _get_nc(use_mask)
    Rm = _rot_mat()

    def swz(a):  # [K, M] -> [p, (u, M)] with K = (u, p)
        k, m = a.shape
        return np.ascontiguousarray(
            a.reshape(KT, P, m).transpose(1, 0, 2).reshape(P, KT * m))

    xT = swz(x.reshape(N, DIM).T.astype(np.float32)).astype(ml_dtypes.bfloat16)
    maskrow = ((mask.reshape(1, N).astype(np.float32)) - 1.0) * NEG_BIG

    cosT = np.cos(rope).T.astype(np.float32)
    sinT = np.sin(rope).T.astype(np.float32)
    cs = np.concatenate([cosT, sinT], axis=1).astype(ml_dtypes.bfloat16)

    # T[d, g, p] = (d == g*DG + p%DG); m512[p, n] = (n%IB == p//DG)
    d_i = np.arange(D)[:, None, None]
    g_i = np.arange(NG)[None, :, None]
    p_i = np.arange(P)[None, None, :]
    tconst = (d_i == g_i * DG + p_i % DG).astype(np.float32)
    tconst = tconst.reshape(D, NG * P).astype(ml_dtypes.float8_e4m3)
    p_2 = np.arange(P)[:, None]
    n_2 = np.arange(N)[None, :]
    m512 = ((n_2 % IB) == (p_2 // DG)).astype(np.float32)
    m512 = m512.astype(ml_dtypes.bfloat16)

    identf = np.eye(P, dtype=np.float32)

    # rel: [h, it, p=(i_l*DG+d_l), (block16, g, j)] fp8, scaled by RELSCALE
    rel8 = (rel_pos[0] * RELSCALE).astype(ml_dtypes.float8_e4m3)
    # [h, (it, b16, i_l), j, (g, d_l)] -> [h, it, i_l, d_l, b16, g, j]
    rel8 = rel8.reshape(H, NT, NBT, IB, N, NG, DG)
    rel8 = np.ascontiguousarray(rel8.transpose(0, 1, 3, 6, 2, 5, 4))
    rel8 = rel8.reshape(H, NT, P, NBT * NG * N)

    in_maps = []
    for h in range(N_CORES):
        sl = slice(h * D, (h + 1) * D)
        wq, wk = Wq[:, sl], Wk[:, sl]
        wqks = np.concatenate([wq, wk, wq @ Rm, wk @ Rm], axis=1)
        biasc = np.stack([bq[sl], bk[sl] * KF, bq[sl] @ Rm,
                          (bk[sl] @ Rm) * KF, bv[sl]], axis=1).astype(np.float32)
        in_maps.append({
            "xt": xT,
            "wqks": swz(wqks).astype(ml_dtypes.bfloat16),
            "wv": swz(np.ascontiguousarray(Wv[:, sl])).astype(ml_dtypes.bfloat16),
            "biasc": np.ascontiguousarray(biasc),
            "maskrow": np.ascontiguousarray(maskrow),
            "wo": np.ascontiguousarray(Wo[sl, :]).astype(ml_dtypes.bfloat16),
            "cs": cs,
            "tconst": tconst,
            "m512": m512,
            "identf": identf,
            "rel": rel8[h],
        })

    from concourse.bass_utils import run_bass_kernel_spmd
    res = run_bass_kernel_spmd(nc, in_maps, list(range(N_CORES)), trace=TRACE)
    globals()["LAST_RESULT"] = res
    out = np.zeros((N, DIM), dtype=np.float32)
    for h in range(N_CORES):
        out += np.asarray(res.results[h]["out"], dtype=np.float32)
    out += bo[None, :]
    return out.reshape(B, N, DIM)


# revision 16
# speedup vs baseline: 1.3086x; 1.0094x over previous
"""Bass/Trainium2 kernel for nn_Attention_66297115181568 (sparse_attention).

Strategy: head-parallel across 8 NeuronCores; core h computes head h
end-to-end and its 64-row slice of the Wo projection. The host sums the
8 partial (512, 512) outputs (the tensor-parallel all-reduce) and adds bo.

v4 (lessons from v1=76us, v2=102us, v3=90us):
  1. rel-term matmuls in fp8 DoubleRow perf mode (measured 216ns per
     [128, 2, 512] matmul at cruise = 2 fp8/partition/cycle, 2x the
     bf16 col-tiled scheme). DoubleRow is ISA-incompatible with PE
     column tiling, so each (block, g-pair) stationary is a full
     128-column [128, 2, 128] window, zero outside its block strip.
  2. IB=8 block geometry (partitions = 8 rows x 16 d): only NG=4
     d-groups -> 4 repl matmuls + 4 strip-muls (v3 had 16+16, a ~13us
     serial prologue chain). Windows for the 16 blocks of a tile are
     overlapped in a [1928]-elem plane per (gp, t, u): window bl starts
     at 120*bl, strip bl at 128*bl, so each window holds exactly its
     own strip (at column 8*bl+c) and shared zeros elsewhere.
  3. qpad zero-fill as f32-bitcast memsets (4B/cycle instead of 1),
     split gpsimd/DVE, emitted first.
  4. DMA: ALL 16 1MB rel pieces on the sync queue (423 GB/s measured;
     v3's split put pieces behind main-loop ACT doorbells -> 87us
     stragglers). Inputs on the scalar queue, emitted first, so the rel
     stream starts at t~9us (queue-up latency) with no input prefix.
     Outputs (bf16) also on the scalar queue.
  5. tile 0 consumes g-pairs as they are built (repl/strip/matmul
     interleave) instead of waiting for all strips.
  6. Softmax: exp(dots/64) with no max-subtraction; the x64 logit scale
     rides in the k-projection activation (KF); exp in 4 column chunks
     so w-transposes/attnV pipeline behind it.
"""

import sys

sys.path.insert(0, "/opt/trn_rl_repo")

from contextlib import ExitStack

import numpy as np
import ml_dtypes

import concourse.bass as bass
import concourse.tile as tile
from concourse import mybir
from concourse.ap import AP

# problem dims (hardcoded per spec)
B, N, DIM, H, D = 1, 512, 512, 8, 64
INNER = H * D
N_CORES = 8
P = 128                 # SBUF partitions
NT = N // P             # 4 row tiles
KT = DIM // P           # 4 contraction tiles for projections
IB = 8                  # i-block rows per partition-group
NBT = P // IB           # 16 blocks per row tile
DG = P // IB            # 16 d's per d-group
NG = D // DG            # 4 d-groups
NGP = NG // 2           # 2 d-group pairs (DoubleRow)
WS = P - IB             # 120: window stride in the packed plane
PW = WS * (NBT - 1) + P  # 1928: packed plane width
SCALE = D ** -0.5
NEG_BIG = 1.0e36
RELSCALE = 64.0         # host scales rel by this before fp8 cast
KF = SCALE * RELSCALE   # folded into the k/krot projection activation

f32 = mybir.dt.float32
bf16 = mybir.dt.bfloat16
fp8 = mybir.dt.float8e4
AX = mybir.AxisListType
ALU = mybir.AluOpType
AF = mybir.ActivationFunctionType
PM = mybir.MatmulPerfMode


def legalize_multi_waits(nc):
    """This walrus build supports only one sync-wait per instruction; hoist
    extra waits onto same-engine NoOps placed immediately before."""
    nid = 0
    for fn in nc.m.functions:
        for bb in fn.blocks:
            new = []
            changed = False
            for inst in bb.instructions:
                si = inst.sync_info
                waits = si.on_wait if si is not None else []
                if len(waits) > 1:
                    for w in waits[:-1]:
                        nop = mybir.InstNoOp(name=f"I-waitfix-{nid}")
                        nid += 1
                        nop.engine = inst.engine
                        nop.sync_info = mybir.SyncInfo(on_wait=[w], on_update=[])
                        new.append(nop)
                    si.on_wait = [waits[-1]]
                    inst.sync_info = si
                    changed = True
                new.append(inst)
            if changed:
                bb.instructions = new


def build_nc(use_mask=True):
    nc = bass.Bass()

    xt_ext = nc.declare_dram_parameter("xt", [P, KT * N], bf16, isOutput=False)
    wqks_ext = nc.declare_dram_parameter("wqks", [P, KT * 4 * D], bf16,
                                         isOutput=False)
    wv_ext = nc.declare_dram_parameter("wv", [P, KT * D], bf16, isOutput=False)
    biasc_ext = nc.declare_dram_parameter("biasc", [D, 5], f32, isOutput=False)
    maskrow_ext = nc.declare_dram_parameter("maskrow", [1, N], f32,
                                            isOutput=False)
    wo_ext = nc.declare_dram_parameter("wo", [D, DIM], bf16, isOutput=False)
    cs_ext = nc.declare_dram_parameter("cs", [D, 2 * N], bf16, isOutput=False)
    tconst_ext = nc.declare_dram_parameter("tconst", [D, NG * P], fp8,
                                           isOutput=False)
    m512_ext = nc.declare_dram_parameter("m512", [P, N], bf16, isOutput=False)
    identf_ext = nc.declare_dram_parameter("identf", [P, P], bf16, isOutput=False)
    # rel stream: [tile, p=(i_l*16+d_l), (block16, g4, j)] fp8; 1MB pieces
    rel_ext = nc.declare_dram_parameter("rel", [NT, P, NBT * NG * N], fp8,
                                        isOutput=False)
    out_ext = nc.declare_dram_parameter("out", [N, DIM], bf16, isOutput=True)

    with tile.TileContext(nc) as tc, ExitStack() as ctx:
        dma = nc.sync      # HWDGE queue 1: the whole rel stream
        dma2 = nc.scalar   # HWDGE queue 2: inputs first, then outputs
        consts = ctx.enter_context(tc.tile_pool(name="consts", bufs=1))
        pro = ctx.enter_context(tc.tile_pool(name="pro", bufs=1))
        relp = ctx.enter_context(tc.tile_pool(name="relp", bufs=1))
        smp = ctx.enter_context(tc.tile_pool(name="smp", bufs=2))
        smallp = ctx.enter_context(tc.tile_pool(name="smallp", bufs=2))
        outp = ctx.enter_context(tc.tile_pool(name="outp", bufs=2))
        op = ctx.enter_context(tc.tile_pool(name="op", bufs=1))
        psA = ctx.enter_context(
            tc.tile_pool(name="psA", bufs=2, space=bass.MemorySpace.PSUM))
        psB = ctx.enter_context(
            tc.tile_pool(name="psB", bufs=2, space=bass.MemorySpace.PSUM))
        psW = ctx.enter_context(
            tc.tile_pool(name="psW", bufs=2, space=bass.MemorySpace.PSUM))
        psV = ctx.enter_context(
            tc.tile_pool(name="psV", bufs=1, space=bass.MemorySpace.PSUM))
        psO = ctx.enter_context(
            tc.tile_pool(name="psO", bufs=1, space=bass.MemorySpace.PSUM))

        # ---- qpad zero-fill FIRST: f32-bitcast memsets split across
        # gpsimd and DVE (both otherwise idle at t0) ----
        qpad = consts.tile([P, NGP, 2, NT, PW], fp8, name="qpad")
        half0 = qpad[:, 0:1].rearrange("p a t u w -> p (a t u w)").bitcast(f32)
        half1 = qpad[:, 1:2].rearrange("p a t u w -> p (a t u w)").bitcast(f32)
        nc.gpsimd.memset(half0, 0.0)
        nc.vector.memset(half1, 0.0)

        # ---- inputs on the sync queue, ahead of the rel stream (v1-proven
        # arrangement: one queue, inputs as the FIFO prefix; the scalar
        # queue carries only outputs so main-loop ACT work never delays a
        # transfer doorbell) ----
        xt_sb = pro.tile([P, KT, N], bf16)
        dma.dma_start(out=xt_sb[:], in_=xt_ext.rearrange("p (u n) -> p u n", u=KT))
        wqks_sb = pro.tile([P, KT, 4 * D], bf16)
        dma.dma_start(out=wqks_sb[:],
                      in_=wqks_ext.rearrange("p (u m) -> p u m", u=KT))
        wv_sb = pro.tile([P, KT, D], bf16)
        dma.dma_start(out=wv_sb[:], in_=wv_ext.rearrange("p (u m) -> p u m", u=KT))
        bias_cols = consts.tile([D, 5], f32)
        dma.dma_start(out=bias_cols[:], in_=biasc_ext[:])
        maskrow_sb = consts.tile([1, N], f32)
        if use_mask:
            dma.dma_start(out=maskrow_sb[:], in_=maskrow_ext[:])
        cs_sb = consts.tile([D, 2, N], bf16)
        dma.dma_start(out=cs_sb[:], in_=cs_ext.rearrange("d (c n) -> d c n", c=2))
        tconst_sb = consts.tile([D, NG, P], fp8)
        dma.dma_start(out=tconst_sb[:],
                      in_=tconst_ext.rearrange("d (g p) -> d g p", g=NG))
        m512_sb = consts.tile([P, N], bf16)
        dma.dma_start(out=m512_sb[:], in_=m512_ext[:])
        identb = consts.tile([P, P], bf16)
        dma.dma_start(out=identb[:], in_=identf_ext[:])
        wo_sb = consts.tile([D, DIM], bf16)
        dma.dma_start(out=wo_sb[:], in_=wo_ext[:])
        ones_sb = consts.tile([1, N], f32)
        nc.vector.memset(ones_sb, 1.0)

        # ---- rel stream: 2MB pieces (2 per row tile), all on the sync
        # queue; the final piece split into two 1MB halves for a short
        # PE tail ----
        rel_tiles = {}
        hpt = NBT * NG * N // 2    # free elems per half-tile piece (8 blocks)
        for it in range(NT):
            for pc in range(2):
                if (it, pc) == (NT - 1, 1):
                    rpa = relp.tile([P, hpt // 2], fp8, name="rel7a")
                    dma.dma_start(out=rpa[:],
                                  in_=rel_ext[it, :, pc * hpt:pc * hpt + hpt // 2])
                    rpb = relp.tile([P, hpt // 2], fp8, name="rel7b")
                    dma.dma_start(out=rpb[:],
                                  in_=rel_ext[it, :, pc * hpt + hpt // 2:(pc + 1) * hpt])
                    rel_tiles[(it, pc)] = (rpa, rpb)
                else:
                    rp = relp.tile([P, hpt], fp8, name=f"rel{it}_{pc}")
                    dma.dma_start(out=rp[:],
                                  in_=rel_ext[it, :, pc * hpt:(pc + 1) * hpt])
                    rel_tiles[(it, pc)] = rp

        def rel_mv(it, bl, gp):
            """moving AP [128, 2, 512] for (tile, block16, g-pair)"""
            pc, bi = divmod(bl, NBT // 2)   # half-tile piece, block-in-piece
            rp = rel_tiles[(it, pc)]
            if isinstance(rp, tuple):
                # halves split blocks 8-11 / 12-15 of tile 3
                rp = rp[0] if bi < 4 else rp[1]
                bi = bi % 4
            off = (bi * NG + 2 * gp) * N
            full = rp[:]
            return AP(full.tensor, full.offset + off,
                      [list(full.ap[0]), [N, 2], [1, N]])

        # ---- projections (transposed): qT, kT, qrotT, krotT, vT ----
        qkT_sb = pro.tile([D, 4, N], bf16)
        for c in range(4):
            ps_c = psA.tile([P, N], f32, tag="big")
            for u in range(KT):
                nc.tensor.matmul(ps_c[0:D, :],
                                 wqks_sb[:, u, c * D:(c + 1) * D],
                                 xt_sb[:, u, :], start=(u == 0), stop=(u == KT - 1))
            nc.scalar.activation(qkT_sb[:, c, :], ps_c[0:D, :], AF.Identity,
                                 bias=bias_cols[:, c:c + 1],
                                 scale=(KF if c in (1, 3) else 1.0))

        ps_v = psA.tile([P, N], f32, tag="big")
        for u in range(KT):
            nc.tensor.matmul(ps_v[0:D, :], wv_sb[:, u, :], xt_sb[:, u, :],
                             start=(u == 0), stop=(u == KT - 1))
        vT_sb = pro.tile([D, N], bf16)
        nc.scalar.activation(vT_sb[:], ps_v[0:D, :], AF.Identity,
                             bias=bias_cols[:, 4:5])

        # ---- RoPE on DVE ----
        qkp_sb = pro.tile([D, 2, N], bf16)
        t1 = pro.tile([D, N], bf16, tag="ropet1")
        t2 = pro.tile([D, N], bf16, tag="ropet2")
        for c in range(2):  # 0: q, 1: k
            nc.vector.tensor_mul(t1[:], qkT_sb[:, c, :], cs_sb[:, 0, :])
            nc.vector.tensor_mul(t2[:], qkT_sb[:, 2 + c, :], cs_sb[:, 1, :])
            nc.vector.tensor_add(qkp_sb[:, c, :], t1[:], t2[:])
        qpT = qkp_sb[:, 0, :]
        kpT = qkp_sb[:, 1, :]

        # ---- v -> [j, d] layout via PE transposes (bf16) ----
        v_sb = pro.tile([P, NT, D], bf16)
        for jt in range(NT):
            pv = psW.tile([P, P], bf16, tag="tp")
            nc.tensor.transpose(pv[:, 0:D], vT_sb[:, jt * P:(jt + 1) * P],
                                identb[0:D, 0:D])
            nc.scalar.copy(v_sb[:, jt, :], pv[:, 0:D])

        # ---- Qpad strips: Rep_g[p, n] = q'T[g*16 + p%16, n]; strips go to
        # plane (gp=g//2, t=g%2): [u, 128*bl + c] (c = p//16 in-strip col),
        # read back as windows [u, 120*bl : 120*bl+128]. ----
        qpad_full = qpad[:]
        ppair = list(qpad_full.ap[0])

        def emit_strip(g):
            ps_rep = psB.tile([P, N], f32, tag="rep")
            nc.tensor.matmul(ps_rep[:], tconst_sb[:, g, :], qpT,
                             start=True, stop=True)
            strip_out = AP(qpad_full.tensor,
                           qpad_full.offset + g * (NT * PW),
                           [ppair, [PW, NT], [P, NBT], [1, IB]])
            nc.vector.tensor_mul(
                strip_out,
                ps_rep.rearrange("p (u v c) -> p u v c", u=NT, v=NBT)[:],
                m512_sb.rearrange("p (u v c) -> p u v c", u=NT, v=NBT)[:])

        # ---- main loop; tile 0 interleaves strip construction ----
        o_tiles = [op.tile([P, DIM], bf16, name=f"o{it}") for it in range(NT)]
        for it in range(NT):
            dots_ps = psA.tile([P, N], f32, tag="big")
            nc.tensor.matmul(dots_ps[:], qpT[:, it * P:(it + 1) * P], kpT,
                             start=True, stop=False, skip_group_check=True)
            if use_mask:
                nc.tensor.matmul(dots_ps[:], ones_sb[:, 0:P], maskrow_sb[:],
                                 start=False, stop=False, skip_group_check=True)
            if it == 0:
                # gp-outer: build strips for pair gp, then sweep its blocks
                for gp in range(NGP):
                    emit_strip(2 * gp)
                    emit_strip(2 * gp + 1)
                    for bl in range(NBT):
                        nc.tensor.matmul(
                            dots_ps[:],
                            qpad[:, gp, :, it, WS * bl:WS * bl + P],
                            rel_mv(it, bl, gp),
                            start=False,
                            stop=(gp == NGP - 1 and bl == NBT - 1),
                            perf_mode=PM.DoubleRow,
                            skip_group_check=True)
            else:
                for bl in range(NBT):
                    for gp in range(NGP):
                        nc.tensor.matmul(
                            dots_ps[:],
                            qpad[:, gp, :, it, WS * bl:WS * bl + P],
                            rel_mv(it, bl, gp),
                            start=False,
                            stop=(bl == NBT - 1 and gp == NGP - 1),
                            perf_mode=PM.DoubleRow,
                            skip_group_check=True)

            # softmax: unnormalized exp(dots/RELSCALE) in bf16, no
            # max-subtraction; 4 column chunks. All 4 transposes run before
            # the attnV matmuls so the PE never stalls on a DVE copy.
            w_sm = smp.tile([P, N], bf16, tag="w_sm")
            rowsum4 = smallp.tile([P, NT], f32, tag="rowsum4")
            wT_sb = outp.tile([P, NT, P], bf16, tag="wT_sb")
            attn_ps = psV.tile([D, P], f32, tag="attn")
            for jt in range(NT):
                nc.scalar.activation(w_sm[:, jt * P:(jt + 1) * P],
                                     dots_ps[:, jt * P:(jt + 1) * P], AF.Exp,
                                     scale=1.0 / RELSCALE,
                                     accum_out=rowsum4[:, jt:jt + 1])
                wp = psW.tile([P, P], bf16, tag="tp")
                nc.tensor.transpose(wp[:], w_sm[:, jt * P:(jt + 1) * P],
                                    identb[:])
                nc.vector.tensor_copy(wT_sb[:, jt, :], wp[:])
            for jt in range(NT):
                nc.tensor.matmul(attn_ps[:], v_sb[:, jt, :], wT_sb[:, jt, :],
                                 start=(jt == 0), stop=(jt == NT - 1))
            rowsum = smallp.tile([P, 1], f32, tag="rowsum")
            nc.vector.tensor_reduce(rowsum[:], rowsum4[:], AX.X, ALU.add)
            rcp = smallp.tile([P, 1], f32, tag="rcp")
            nc.vector.reciprocal(rcp[:], rowsum[:])

            attn_sb = outp.tile([D, P], bf16, tag="attn_sb")
            nc.scalar.copy(attn_sb[:], attn_ps[:])
            out_ps = psO.tile([P, DIM], f32, tag="out")
            nc.tensor.matmul(out_ps[:], attn_sb[:], wo_sb[:], start=True, stop=True)
            nc.scalar.activation(o_tiles[it][:], out_ps[:], AF.Copy, scale=rcp[:])
            dma2.dma_start(out=out_ext[it * P:(it + 1) * P, :], in_=o_tiles[it][:])

    legalize_multi_waits(nc)
    return nc


_NC_CACHE = None
TRACE = False
LAST_RESULT = None


def _get_nc(use_mask):
    global _NC_CACHE
    if _NC_CACHE is None or _NC_CACHE[1] != use_mask:
        _NC_CACHE = (build_nc(use_mask), use_mask)
    return _NC_CACHE[0]


def _rot_mat():
    """rotate_half as a right-multiply matrix: rot(q) = q @ Rm."""
    Rm = np.zeros((D, D), np.float32)
    for i in range(D // 2):
        Rm[2 * i + 1, 2 * i] = -1.0
        Rm[2 * i, 2 * i + 1] = 1.0
    return Rm


def kernel(**inputs):
    x = np.asarray(inputs["x"], dtype=np.float32)
    mask = np.asarray(inputs["mask"])
    rope = np.asarray(inputs["rope"], dtype=np.float32)
    rel_pos = np.asarray(inputs["rel_pos"], dtype=np.float32)
    Wq = np.asarray(inputs["Wq"], dtype=np.float32)
    bq = np.asarray(inputs["bq"], dtype=np.float32)
    Wk = np.asarray(inputs["Wk"], dtype=np.float32)
    bk = np.asarray(inputs["bk"], dtype=np.float32)
    Wv = np.asarray(inputs["Wv"], dtype=np.float32)
    bv = np.asarray(inputs["bv"], dtype=np.float32)
    Wo = np.asarray(inputs["Wo"], dtype=np.float32)
    bo = np.asarray(inputs["bo"], dtype=np.float32)

    use_mask = not bool(np.asarray(mask).all())
    nc = _get_nc(use_mask)
    Rm = _rot_mat()

    def swz(a):  # [K, M] -> [p, (u, M)] with K = (u, p)
        k, m = a.shape
        return np.ascontiguousarray(
            a.reshape(KT, P, m).transpose(1, 0, 2).reshape(P, KT * m))

    xT = swz(x.reshape(N, DIM).T.astype(np.float32)).astype(ml_dtypes.bfloat16)
    maskrow = ((mask.reshape(1, N).astype(np.float32)) - 1.0) * NEG_BIG

    cosT = np.cos(rope).T.astype(np.float32)
    sinT = np.sin(rope).T.astype(np.float32)
    cs = np.concatenate([cosT, sinT], axis=1).astype(ml_dtypes.bfloat16)

    # T[d, g, p] = (d == g*DG + p%DG); m512[p, n] = (n%IB == p//DG)
    d_i = np.arange(D)[:, None, None]
    g_i = np.arange(NG)[None, :, None]
    p_i = np.arange(P)[None, None, :]
    tconst = (d_i == g_i * DG + p_i % DG).astype(np.float32)
    tconst = tconst.reshape(D, NG * P).astype(ml_dtypes.float8_e4m3)
    p_2 = np.arange(P)[:, None]
    n_2 = np.arange(N)[None, :]
    m512 = ((n_2 % IB) == (p_2 // DG)).astype(np.float32)
    m512 = m512.astype(ml_dtypes.bfloat16)

    identf = np.eye(P, dtype=np.float32).astype(ml_dtypes.bfloat16)

    # rel: [h, it, p=(i_l*DG+d_l), (block16, g, j)] fp8, scaled by RELSCALE
    rel8 = (rel_pos[0] * RELSCALE).astype(ml_dtypes.float8_e4m3)
    # [h, (it, b16, i_l), j, (g, d_l)] -> [h, it, i_l, d_l, b16, g, j]
    rel8 = rel8.reshape(H, NT, NBT, IB, N, NG, DG)
    rel8 = np.ascontiguousarray(rel8.transpose(0, 1, 3, 6, 2, 5, 4))
    rel8 = rel8.reshape(H, NT, P, NBT * NG * N)

    in_maps = []
    for h in range(N_CORES):
        sl = slice(h * D, (h + 1) * D)
        wq, wk = Wq[:, sl], Wk[:, sl]
        wqks = np.concatenate([wq, wk, wq @ Rm, wk @ Rm], axis=1)
        biasc = np.stack([bq[sl], bk[sl] * KF, bq[sl] @ Rm,
                          (bk[sl] @ Rm) * KF, bv[sl]], axis=1).astype(np.float32)
        in_maps.append({
            "xt": xT,
            "wqks": swz(wqks).astype(ml_dtypes.bfloat16),
            "wv": swz(np.ascontiguousarray(Wv[:, sl])).astype(ml_dtypes.bfloat16),
            "biasc": np.ascontiguousarray(biasc),
            "maskrow": np.ascontiguousarray(maskrow),
            "wo": np.ascontiguousarray(Wo[sl, :]).astype(ml_dtypes.bfloat16),
            "cs": cs,
            "tconst": tconst,
            "m512": m512,
            "identf": identf,
            "rel": rel8[h],
        })

    from concourse.bass_utils import run_bass_kernel_spmd
    res = run_bass_kernel_spmd(nc, in_maps, list(range(N_CORES)), trace=TRACE)
    globals()["LAST_RESULT"] = res
    out = np.zeros((N, DIM), dtype=np.float32)
    for h in range(N_CORES):
        out += np.asarray(res.results[h]["out"], dtype=np.float32)
    out += bo[None, :]
    return out.reshape(B, N, DIM)
